# revision 24
# baseline (speedup 1.0000x reference)
"""Multi-head causal attention (B=2, T=2048, E=768, H=12, D=64) on 8 trn2 cores.

Sharding: core c handles batch b=c//4 and heads [3g, 3g+1, 3g+2] (g=c%4).
Each core computes its 3 heads' attention plus their partial contribution to
the final projection; the host sums the 4 partials per batch.

Differences vs the original transpose-based kernel:
- V is projected directly in [tokens, dims] orientation (lhsT = x token
  tile), eliminating all 48 PE transposes (-12k PE cycles/body); its bias is
  folded into a host-side constant row (softmax weights sum to 1).
- Unified per-head attention over key-block PAIRS, bank-aligned: full pair
  tiles are [block 2p | block 2p+1] (1024 cols, ONE exp); the 4 diagonal
  blocks A..D of each quarter pack into two tiles:
    st1 (2 banks): [A-main 384 | A-diag 128 | B 384 | C-solo 128]
    st2 (1 bank):  [C-main 128 | D 128]
  Every matmul output stays inside a 2KB psum bank (hard hw rule) and each
  tile needs one exp (60 exps/body).  Diagonal 128-blocks are masked AFTER
  exp by gpsimd affine_select on the SBUF pt tiles.
- FLIPPED PV: the exp'd score chunks ([128 keys, 128 queries]) are the
  STATIONARY operand and v_all [128, 65] streams, accumulating O^T per
  query-chunk into psum [128 queries, 4, 65|l].  PV streams 65 cols per
  (block, chunk) = the output-element bound: 8,840 cycles/head vs 17,408
  for the classic orientation (which is moving-column-bound on pt).
  Bonus: l lands on the PARTITION axis, so normalize is a plain DVE
  reciprocal + per-partition tensor_scalar (the whole PE-broadcast /
  copy machinery is gone); PE transposes (16 tiles x 3 heads) restore
  dim-major ot01/ot2 for the output projection.
- normalize(q) + phase3(q) are deferred into step q+1 AFTER its projection
  chunk, so their dependency chain hides behind ~6k cycles of independent
  projection matmuls in the in-order PE queue.
- Output partials are DMA'd as fp16 (halves output HBM traffic; summed in
  f64 on the host).

Everything numeric is bf16 into f32 psum.  fp8(e4m3) + DoubleRow was built
and measured: it passes BIR/hardware fine (see git-less probe history) but
any fp8 tensor in the q/k path, v, or even just the output projection blows
the rel-2e-2 max-norm budget (exp amplifies score jitter; peaked softmax rows
expose raw v quantization; max over 3.1M outputs sits ~5 sigma up), so it is
not used.

Measured: ~118us/body (vs 171us for the classic-orientation kernel).
PE-cycle-bound; ~165k PE cycles/body (proj 55k, QK 52k, PV 26.5k,
transposes 6k, out-proj 25k).  KDUP=q probes measured ~1.13GHz marginal in
the OLD long-stream regime, yet this kernel beats the 1.2GHz pure-stream
bound -- the clock gate rewards the flipped PV's short clean streams, so
treat clock models as advisory and measure.  ACT (exp, 55us) and DVE have
slack.  Row-tile co-execution does NOT exist (probe_rows.py: packed/serial
= 0.73, just overhead hiding), and fp8/DoubleRow is numerically dead at
this tolerance everywhere.

PSUM banks (8): stA0/stA1 [128,1024] f32 (2 each) + otl0/1/2 [128,512] + bcp.
`repeat` unrolls the body N times in one NEFF; test.py measures per-body HW
time as the slope of wall time vs N.  KDUP=q/e/2 add duplicate work for
differential load probing (default off).
"""
import numpy as np

EMBED_DIM = 768
B = 2
T = 2048
N_CORES = 8
NT = T // 128           # 16 key/query tiles
SCALE = 1.0 / np.sqrt(64.0)

_state = {}


def _build(repeat=1):
    import concourse.tile as tile
    from concourse import bacc, mybir
    from concourse.masks import make_identity

    F32 = mybir.dt.float32
    F32R = mybir.dt.float32r
    BF16 = mybir.dt.bfloat16
    FP8 = mybir.dt.float8e4
    F16 = mybir.dt.float16

    nc = bacc.Bacc("TRN2", target_bir_lowering=False, debug=False)

    xT_d = nc.dram_tensor("xT", [EMBED_DIM, T], BF16, kind="ExternalInput").ap()
    # columns ordered [q0 q1 | k0 k1 | q2 | k2]
    wqk_d = nc.dram_tensor("wqk", [EMBED_DIM, 384], BF16, kind="ExternalInput").ap()
    wv_d = nc.dram_tensor("wv", [EMBED_DIM, 192], BF16, kind="ExternalInput").ap()
    bqk_d = nc.dram_tensor("bqk", [384, 1], F32, kind="ExternalInput").ap()
    # w_final^T rows for this core's 192 dims: [0:128) and [128:192)
    wf_d = nc.dram_tensor("wf", [192, EMBED_DIM], BF16, kind="ExternalInput").ap()
    out_d = nc.dram_tensor("out_p", [T, EMBED_DIM], F16, kind="ExternalOutput").ap()

    with tile.TileContext(nc) as tc:
        with tc.tile_pool(name="const", bufs=1) as const, \
             tc.tile_pool(name="persist", bufs=1) as persist, \
             tc.tile_pool(name="sbod", bufs=1) as sbp, \
             tc.tile_pool(name="psod", bufs=1, space="PSUM") as psp:
            # ---- constants ----
            wqk_sb = const.tile([128, 6, 384], BF16)
            wv_sb = const.tile([128, 6, 192], BF16)
            nc.sync.dma_start(out=wqk_sb[:], in_=wqk_d.rearrange("(k p) c -> p k c", p=128))
            nc.gpsimd.dma_start(out=wv_sb[:], in_=wv_d.rearrange("(k p) c -> p k c", p=128))
            bqk_sb = [const.tile([128, 1], F32, name=f"bqk{m}", tag=f"bqk{m}")
                      for m in range(3)]
            for m in range(3):
                nc.sync.dma_start(out=bqk_sb[m][:], in_=bqk_d[128 * m:128 * (m + 1), :])
            wf01_sb = const.tile([128, EMBED_DIM], BF16)
            wf2_sb = const.tile([64, EMBED_DIM], BF16)
            nc.gpsimd.dma_start(out=wf01_sb[:], in_=wf_d[0:128, :])
            nc.gpsimd.dma_start(out=wf2_sb[:], in_=wf_d[128:192, :])
            ident_f = const.tile([128, 128], F32)
            make_identity(nc, ident_f)
            ident_b = const.tile([128, 128], BF16)
            nc.vector.tensor_copy(out=ident_b[:], in_=ident_f[:])

            # ---- persistent activations ----
            qA = persist.tile([128, T], BF16)   # q0 @0:64, q1 @64:128
            kA = persist.tile([128, T], BF16)   # k0 @0:64, k1 @64:128
            qB = persist.tile([64, T], BF16)    # q2
            kB = persist.tile([64, T], BF16)    # k2
            # [v | 1] per (key tile, head).  Double-buffered by body parity
            # (the next body's projection writes them while this body's
            # final-quarter PVs still read).
            v_alls = [persist.tile([128, NT, 3, 66], BF16, name=f"v_all{p}",
                                   tag=f"v_all{p}") for p in range(2)]
            for p in range(2):
                nc.vector.memset(v_alls[p][:], 1.0)   # col 64 stays 1.0
            ot01 = persist.tile([128, T], BF16)  # heads 0 (@0:64) & 1 (@64:128)
            ot2 = persist.tile([64, T], BF16)    # head 2

            env = locals()
            bodies = [_make_body(nc, tc, rep, env) for rep in range(repeat)]
            bodies[0][0]()
            for rep in range(repeat):
                nxt = bodies[rep + 1][0] if rep + 1 < repeat else None
                bodies[rep][1](nxt)

    nc.compile()
    return nc


def _make_body(nc, tc, rep, env):
    """Build one body's emission closures; returns (prologue, main)."""
    import os
    from concourse import mybir
    kdup = os.environ.get("KDUP", "")

    F32 = mybir.dt.float32
    F32R = mybir.dt.float32r
    BF16 = mybir.dt.bfloat16
    F16 = mybir.dt.float16
    Exp = mybir.ActivationFunctionType.Exp
    MULT = mybir.AluOpType.mult
    GE = mybir.AluOpType.is_ge
    DR = mybir.MatmulPerfMode.DoubleRow

    xT_d, out_d = env["xT_d"], env["out_d"]
    wqk_sb, wv_sb = env["wqk_sb"], env["wv_sb"]
    bqk_sb = env["bqk_sb"]
    wf01_sb, wf2_sb = env["wf01_sb"], env["wf2_sb"]
    ident_b = env["ident_b"]
    qA, kA, qB, kB = env["qA"], env["kA"], env["qB"], env["kB"]
    v_all = env["v_alls"][rep % 2]
    ot01, ot2 = env["ot01"], env["ot2"]
    dmae = [nc.sync, nc.gpsimd]

    sbp, psp = env["sbp"], env["psp"]

    # ---- input DMA: xT as 4 chunks of [128, 6, 512] ----
    xT_t = [sbp.tile([128, 6, 512], BF16, name=f"xT{rep}_{n}", tag=f"xT{n}")
            for n in range(4)]

    def emit_xt_dma():
        for n in range(4):
            for k in range(6):
                nc.sync.dma_start(
                    out=xT_t[n][:, k, :],
                    in_=xT_d[128 * k:128 * (k + 1), 512 * n:512 * (n + 1)])

    gidx = [0]

    def st_tile(shape, name):
        t = psp.tile(shape, F32, name=name, tag=f"stA{gidx[0] % 2}")
        gidx[0] += 1
        return t

    def qk_group(m, n):
        # m: 0=[q0q1]->qA, 1=[k0k1]->kA, 2=[q2|k2]->qB+kB
        c0, c1 = 128 * m, 128 * (m + 1)
        ps = st_tile([128, 512], f"pg{rep}_{m}{n}")
        for k in range(6):
            nc.tensor.matmul(ps[:], lhsT=wqk_sb[:, k, c0:c1],
                             rhs=xT_t[n][:, k, :], start=(k == 0), stop=(k == 5))
        nsl = slice(512 * n, 512 * (n + 1))
        if m < 2:
            dst = qA if m == 0 else kA
            nc.vector.tensor_scalar_add(out=dst[:, nsl], in0=ps[:],
                                        scalar1=bqk_sb[m][:])
        else:
            nc.vector.tensor_scalar_add(out=qB[:, nsl], in0=ps[0:64, :],
                                        scalar1=bqk_sb[2][0:64, :])
            nc.vector.tensor_scalar_add(out=kB[:, nsl], in0=ps[64:128, :],
                                        scalar1=bqk_sb[2][64:128, :])

    def v_group(ii, n):
        # token tile t = 4n+ii: v^T(t) = x_tile @ wv -> [128 tokens, 192]
        t = 4 * n + ii
        ps = st_tile([128, 192], f"pv{rep}_{t}")
        for k in range(6):
            nc.tensor.matmul(ps[:], lhsT=xT_t[n][:, k, 128 * ii:128 * (ii + 1)],
                             rhs=wv_sb[:, k, :], start=(k == 0), stop=(k == 5))
        nc.vector.tensor_copy(
            out=v_all[:, t, :, 0:64],
            in_=ps[:].rearrange("p (h d) -> p h d", h=3))

    def proj_chunk(n):
        for m in range(3):
            qk_group(m, n)
        for ii in range(4):
            v_group(ii, n)

    # ---- attention ----
    def KQ(h):
        if h == 0:
            return kA[0:64], qA[0:64]
        if h == 1:
            return kA[64:128], qA[64:128]
        return kB[:], qB[:]

    pts = {}    # (h, q, p) -> pt tile
    pvps = {}   # q -> [per-head psum tile [128, 4, 128] f32 (qc, dims|l)]
    pvst = {}   # (q, h) -> started flag for psum zero-region

    def diag_mask(pt, c0):
        # zero pt[:, c0:c0+128] where key row p > query col offset c
        nc.gpsimd.affine_select(
            out=pt[:, c0:c0 + 128], in_=pt[:, c0:c0 + 128],
            compare_op=GE, fill=0.0,
            base=0, channel_multiplier=-1, pattern=[[1, 128]])

    def qk_unit(h, qq, p):
        """Score tile + exp for unit p of quarter qq, head h."""
        Kh, Qh = KQ(h)
        base = 512 * qq
        nfull = 2 * qq

        def kblk(i):
            return Kh[:, 128 * i:128 * (i + 1)]

        def qrng(a, b):
            return Qh[:, base + a:base + b]

        if p < nfull:
            # full pair: key blocks 2p, 2p+1, all 512 queries
            st = st_tile([128, 1024], f"st{rep}_{h}{qq}{p}")
            if "2" in kdup and h < 2:
                # probe: same matmul on the OPPOSITE row half, spare bank
                Kf, Qf = KQ(1 - h)
                dps = psp.tile([128, 512], F32, name=f"dp{rep}_{h}{qq}{p}",
                               tag="bcp")
                nc.tensor.matmul(dps[:], lhsT=Kf[:, 128 * 2 * p:128 * (2 * p + 1)],
                                 rhs=Qf[:, 512 * qq:512 * qq + 512],
                                 start=True, stop=True, skip_group_check=True)
            for _ in range(2 if "q" in kdup else 1):
                nc.tensor.matmul(st[:, 0:512], lhsT=kblk(2 * p), rhs=qrng(0, 512),
                                 start=True, stop=True, skip_group_check=True)
                nc.tensor.matmul(st[:, 512:1024], lhsT=kblk(2 * p + 1),
                                 rhs=qrng(0, 512), start=True, stop=True,
                                 skip_group_check=True)
            pt = sbp.tile([128, 1024], BF16, name=f"pF{rep}_{h}{qq}{p}",
                          tag=f"pF{h}_{qq % 2}_{p}")
            nc.scalar.activation(out=pt[:], in_=st[:], func=Exp,
                                 scale=float(SCALE))
            if "e" in kdup:
                dmy = sbp.tile([128, 1024], BF16, name=f"dm{rep}_{h}{qq}{p}",
                               tag=f"dm{p % 2}")
                nc.scalar.activation(out=dmy[:], in_=st[:], func=Exp,
                                     scale=float(SCALE))
            pts[(h, qq, p)] = pt
        elif p == nfull:
            # D1: bank A = [A-main 384 | A-diag 128], bank B = [B 384 | C-solo 128]
            iA = 4 * qq
            st = st_tile([128, 1024], f"sd{rep}_{h}{qq}")
            nc.tensor.matmul(st[:, 0:384], lhsT=kblk(iA), rhs=qrng(128, 512),
                             start=True, stop=False)
            nc.tensor.matmul(st[:, 384:512], lhsT=kblk(iA), rhs=qrng(0, 128),
                             start=False, stop=True)
            nc.tensor.matmul(st[:, 512:896], lhsT=kblk(iA + 1), rhs=qrng(128, 512),
                             start=True, stop=False)
            nc.tensor.matmul(st[:, 896:1024], lhsT=kblk(iA + 2), rhs=qrng(256, 384),
                             start=False, stop=True)
            pt = sbp.tile([128, 1024], BF16, name=f"pD1{rep}_{h}{qq}",
                          tag=f"pD1{h}_{qq % 2}")
            nc.scalar.activation(out=pt[:], in_=st[:], func=Exp,
                                 scale=float(SCALE))
            diag_mask(pt, 384)        # A's diagonal (queries 0:128)
            diag_mask(pt, 512)        # B's diagonal (queries 128:256)
            diag_mask(pt, 896)        # C's diagonal (queries 256:384)
            pts[(h, qq, p)] = pt
        else:
            # D2: [C-main 128 | D 128]
            iA = 4 * qq
            st = st_tile([128, 256], f"s2{rep}_{h}{qq}")
            nc.tensor.matmul(st[:, 0:128], lhsT=kblk(iA + 2), rhs=qrng(384, 512),
                             start=True, stop=False)
            nc.tensor.matmul(st[:, 128:256], lhsT=kblk(iA + 3), rhs=qrng(384, 512),
                             start=False, stop=True)
            pt = sbp.tile([128, 256], BF16, name=f"pD2{rep}_{h}{qq}",
                          tag=f"pD2{h}_{qq % 2}")
            nc.scalar.activation(out=pt[:], in_=st[:], func=Exp,
                                 scale=float(SCALE))
            diag_mask(pt, 128)        # D's diagonal (queries 384:512)
            pts[(h, qq, p)] = pt

    def pv_unit(h, pvq, p):
        """Flipped PV: pt chunks are STATIONARY ([128 keys, 128 queries]),
        v_all [128, 65] streams -> psum [128 queries, 4qc, 65] per head.
        Streams 65 cols/block-chunk instead of up-to-512: PV runs at the
        output-element bound (8,840 cycles/head vs 17,408)."""
        pvp = pvps[pvq][h]
        nfull = 2 * pvq
        pt = pts.pop((h, pvq, p))

        def mm(qc, c0, i, stop=False):
            st = not pvst.get((pvq, h), False)
            pvst[(pvq, h)] = True
            nc.tensor.matmul(pvp[:, qc, 0:65], lhsT=pt[:, c0:c0 + 128],
                             rhs=v_all[:, i, h, 0:65], start=st, stop=stop)

        if p < nfull:
            for j in range(2):
                for qc in range(4):
                    mm(qc, 512 * j + 128 * qc, 2 * p + j)
        elif p == nfull:
            iA = 4 * pvq
            for qc in (1, 2, 3):
                mm(qc, 128 * (qc - 1), iA)              # A-main
            mm(0, 384, iA)                              # A-diag
            for qc in (1, 2, 3):
                mm(qc, 512 + 128 * (qc - 1), iA + 1)    # B
            mm(2, 896, iA + 2)                          # C-solo
        else:
            iA = 4 * pvq
            mm(3, 0, iA + 2)                            # C-main
            mm(3, 128, iA + 3, stop=True)               # D

    def normalize(q):
        """l sits on the 65th column per query-partition: reciprocal +
        per-partition scale on DVE, then PE-transpose back to dim-major
        ot01/ot2 for the output projection."""
        pvp = pvps.pop(q)
        otns, rcs = [], []
        for qc in range(4):
            t = 4 * q + qc
            rc = sbp.tile([128, 3], F32, name=f"rc{rep}_{t}", tag=f"rc{qc}")
            otn = sbp.tile([128, 3, 64], BF16, name=f"on{rep}_{t}",
                           tag=f"on{qc}")
            for h in range(3):
                with nc.allow_low_precision(reason="f32r recip"):
                    nc.vector.reciprocal(out=rc[:, h:h + 1],
                                         in_=pvp[h][:, qc, 64:65])
            rcs.append(rc)
            otns.append(otn)
        for qc in range(4):
            for h in range(3):
                nc.vector.tensor_scalar_mul(out=otns[qc][:, h, :],
                                            in0=pvp[h][:, qc, 0:64],
                                            scalar1=rcs[qc][:, h:h + 1])
        for qc in range(4):
            t = 4 * q + qc
            tsl = slice(128 * t, 128 * (t + 1))
            dsts = [ot01[0:64, tsl], ot01[64:128, tsl], ot2[0:64, tsl]]
            for h in range(3):
                tp = psp.tile([64, 128], BF16, name=f"tp{rep}_{t}{h}",
                              tag=f"pv{h}")
                nc.tensor.transpose(tp[:], otns[qc][:, h, :], ident_b[:])
                nc.vector.tensor_copy(out=dsts[h], in_=tp[:])

    def phase3_tile(i):
        # out[128i:128i+128, :] = [ot01; ot2][:, tile i].T @ wf
        fpa = psp.tile([128, 512], F32, name=f"fpa{rep}_{i}", tag="bcp")
        fpb = psp.tile([128, 256], F32, name=f"fpb{rep}_{i}",
                       tag=f"pv{i % 3}")
        csl = slice(128 * i, 128 * (i + 1))
        for (fp, n0, n1) in [(fpa, 0, 512), (fpb, 512, 768)]:
            nc.tensor.matmul(fp[:, 0:n1 - n0], lhsT=ot01[:, csl],
                             rhs=wf01_sb[:, n0:n1], start=True, stop=False)
            nc.tensor.matmul(fp[:, 0:n1 - n0], lhsT=ot2[:, csl],
                             rhs=wf2_sb[:, n0:n1], start=False, stop=True)
        ob = sbp.tile([128, EMBED_DIM], F16, name=f"ob{rep}_{i}",
                      tag=f"ob{i % 3}")
        nc.vector.tensor_copy(out=ob[:, 0:512], in_=fpa[:])
        nc.vector.tensor_copy(out=ob[:, 512:768], in_=fpb[:])
        dmae[i % 2].dma_start(out=out_d[128 * i:128 * (i + 1), :], in_=ob[:])

    # ---- emission closures ----
    def prologue():
        emit_xt_dma()
        proj_chunk(0)
        for p in range(2):          # quarter 0 bootstrap: D1, D2
            for h in range(3):
                qk_unit(h, 0, p)

    def main(next_prologue=None):
        # normalize(q)/phase3(q) are deferred into step q+1, AFTER its
        # projection chunk: the proj matmuls sit ahead of the recip/bcast
        # chain in the in-order PE queue, hiding the chain's DVE latency.
        pending = None
        for q in range(4):
            if q == 3 and next_prologue is not None:
                next_prologue()
            if q < 3:
                proj_chunk(q + 1)
            if pending is not None:
                normalize(pending)
                for i in range(4 * pending, 4 * pending + 4):
                    phase3_tile(i)
            qq = q + 1 if q < 3 else None
            pvps[q] = [psp.tile([128, 4, 128], F32, name=f"pv{rep}_{q}{h}",
                                tag=f"pv{h}") for h in range(3)]
            nqk = 2 * (q + 1) + 2 if qq is not None else 0
            npv = 2 * q + 2
            for p in range(max(nqk, npv)):
                for h in range(3):
                    if p < nqk:
                        qk_unit(h, qq, p)
                    if p < npv:
                        pv_unit(h, q, p)
            pending = q
        normalize(3)
        for i in range(12, 16):
            phase3_tile(i)

    return prologue, main


def _prep_inputs(x, w_qkv, b_qkv, w_final):
    """Build the 8 per-core input maps from the full inputs."""
    import ml_dtypes
    fp8 = ml_dtypes.float8_e4m3
    bf16 = ml_dtypes.bfloat16

    x = np.asarray(x, dtype=np.float32)
    w_qkv = np.asarray(w_qkv, dtype=np.float32)
    b_qkv = np.asarray(b_qkv, dtype=np.float32)
    w_final = np.asarray(w_final, dtype=np.float32)
    E = EMBED_DIM

    in_maps = []
    for c in range(N_CORES):
        b = c // 4
        g = c % 4
        heads = [3 * g, 3 * g + 1, 3 * g + 2]
        hr = [np.arange(64 * h, 64 * h + 64) for h in heads]
        # [q0 q1 | k0 k1 | q2 | k2]
        rows_qk = np.concatenate([hr[0], hr[1], E + hr[0], E + hr[1], hr[2], E + hr[2]])
        rows_v = np.concatenate(hr) + 2 * E
        xT = np.ascontiguousarray(x[b].T).astype(bf16)               # [768, 2048]
        wqk = np.ascontiguousarray(w_qkv[rows_qk].T).astype(bf16)    # [768, 384]
        wv = np.ascontiguousarray(w_qkv[rows_v].T).astype(bf16)      # [768, 192]
        bqk = np.ascontiguousarray(b_qkv[rows_qk][:, None])
        wf = np.ascontiguousarray(w_final[:, np.concatenate(hr)].T).astype(bf16)
        in_maps.append({"xT": xT, "wqk": wqk, "wv": wv, "bqk": bqk, "wf": wf})
    return in_maps


def kernel(x, w_qkv, b_qkv, w_final, _trace=False):
    from concourse.bass_utils import run_bass_kernel_spmd

    if "nc" not in _state:
        _state["nc"] = _build()
    nc = _state["nc"]

    in_maps = _prep_inputs(x, w_qkv, b_qkv, w_final)
    res = run_bass_kernel_spmd(nc, in_maps, list(range(N_CORES)), trace=_trace)
    _state["last_result"] = res

    w_final = np.asarray(w_final, dtype=np.float64)
    b_qkv = np.asarray(b_qkv, dtype=np.float64)
    # v bias folds into a constant row: softmax weights sum to 1
    const_row = w_final @ b_qkv[2 * EMBED_DIM:]

    out = np.empty((B, T, EMBED_DIM), dtype=np.float32)
    for b in range(B):
        acc = np.zeros((T, EMBED_DIM), dtype=np.float64)
        for g in range(4):
            acc += res.results[4 * b + g]["out_p"].astype(np.float64)
        out[b] = (acc + const_row).astype(np.float32)
    return out


# revision 26
# speedup vs baseline: 1.0831x; 1.0831x over previous
"""Multi-head causal attention (B=2, T=2048, E=768, H=12, D=64) on 8 trn2 cores.

Sharding: core c handles batch b=c//4 and heads [3g, 3g+1, 3g+2] (g=c%4).
Each core computes its 3 heads' attention plus their partial contribution to
the final projection; the host sums the 4 partials per batch.

Differences vs the original transpose-based kernel:
- V is projected directly in [tokens, dims] orientation (lhsT = x token
  tile), eliminating all 48 PE transposes (-12k PE cycles/body); its bias is
  folded into a host-side constant row (softmax weights sum to 1).
- Unified per-head attention over key-block PAIRS, bank-aligned: full pair
  tiles are [block 2p | block 2p+1] (1024 cols, ONE exp); the 4 diagonal
  blocks A..D of each quarter pack into two tiles:
    st1 (2 banks): [A-main 384 | A-diag 128 | B 384 | C-solo 128]
    st2 (1 bank):  [C-main 128 | D 128]
  Every matmul output stays inside a 2KB psum bank (hard hw rule) and each
  tile needs one exp (60 exps/body).  Diagonal 128-blocks are masked AFTER
  exp by gpsimd affine_select on the SBUF pt tiles.
- FLIPPED PV: the exp'd score chunks ([128 keys, 128 queries]) are the
  STATIONARY operand and v_all [128, 65] streams, accumulating O^T per
  query-chunk into psum [128 queries, 4, 65|l].  PV streams 65 cols per
  (block, chunk) = the output-element bound: 8,840 cycles/head vs 17,408
  for the classic orientation (which is moving-column-bound on pt).
  Bonus: l lands on the PARTITION axis, so normalize is a plain DVE
  reciprocal + per-partition tensor_scalar (the whole PE-broadcast /
  copy machinery is gone); PE transposes (16 tiles x 3 heads) restore
  dim-major ot01/ot2 for the output projection.
- normalize(q) + phase3(q) are deferred into step q+1 AFTER its projection
  chunk, so their dependency chain hides behind ~6k cycles of independent
  projection matmuls in the in-order PE queue.
- Output partials are DMA'd as fp16 (halves output HBM traffic; summed in
  f64 on the host).

Everything numeric is bf16 into f32 psum.  fp8(e4m3) + DoubleRow was built
and measured: it passes BIR/hardware fine (see git-less probe history) but
any fp8 tensor in the q/k path, v, or even just the output projection blows
the rel-2e-2 max-norm budget (exp amplifies score jitter; peaked softmax rows
expose raw v quantization; max over 3.1M outputs sits ~5 sigma up), so it is
not used.

Measured: ~118us/body (vs 171us for the classic-orientation kernel).
PE-cycle-bound; ~165k PE cycles/body (proj 55k, QK 52k, PV 26.5k,
transposes 6k, out-proj 25k).  KDUP=q probes measured ~1.13GHz marginal in
the OLD long-stream regime, yet this kernel beats the 1.2GHz pure-stream
bound -- the clock gate rewards the flipped PV's short clean streams, so
treat clock models as advisory and measure.  ACT (exp, 55us) and DVE have
slack.  Row-tile co-execution does NOT exist (probe_rows.py: packed/serial
= 0.73, just overhead hiding), and fp8/DoubleRow is numerically dead at
this tolerance everywhere.

PSUM banks (8): stA0/stA1 [128,1024] f32 (2 each) + otl0/1/2 [128,512] + bcp.
`repeat` unrolls the body N times in one NEFF; test.py measures per-body HW
time as the slope of wall time vs N.  KDUP=q/e/2 add duplicate work for
differential load probing (default off).
"""
import numpy as np

EMBED_DIM = 768
B = 2
T = 2048
N_CORES = 8
NT = T // 128           # 16 key/query tiles
SCALE = 1.0 / np.sqrt(64.0)

_state = {}


def _build(repeat=1):
    import concourse.tile as tile
    from concourse import bacc, mybir
    from concourse.masks import make_identity

    F32 = mybir.dt.float32
    F32R = mybir.dt.float32r
    BF16 = mybir.dt.bfloat16
    FP8 = mybir.dt.float8e4
    F16 = mybir.dt.float16

    nc = bacc.Bacc("TRN2", target_bir_lowering=False, debug=False)

    xT_d = nc.dram_tensor("xT", [EMBED_DIM, T], BF16, kind="ExternalInput").ap()
    # columns ordered [q0 q1 | k0 k1 | q2 | k2]
    wqk_d = nc.dram_tensor("wqk", [EMBED_DIM, 384], BF16, kind="ExternalInput").ap()
    wv_d = nc.dram_tensor("wv", [EMBED_DIM, 192], BF16, kind="ExternalInput").ap()
    bqk_d = nc.dram_tensor("bqk", [384, 1], F32, kind="ExternalInput").ap()
    # w_final^T rows for this core's 192 dims: [0:128) and [128:192)
    wf_d = nc.dram_tensor("wf", [192, EMBED_DIM], BF16, kind="ExternalInput").ap()
    out_d = nc.dram_tensor("out_p", [T, EMBED_DIM], F16, kind="ExternalOutput").ap()

    with tile.TileContext(nc) as tc:
        with tc.tile_pool(name="const", bufs=1) as const, \
             tc.tile_pool(name="persist", bufs=1) as persist, \
             tc.tile_pool(name="sbod", bufs=1) as sbp, \
             tc.tile_pool(name="psod", bufs=1, space="PSUM") as psp:
            # ---- constants ----
            wqk_sb = const.tile([128, 6, 384], BF16)
            wv_sb = const.tile([128, 6, 192], BF16)
            nc.sync.dma_start(out=wqk_sb[:], in_=wqk_d.rearrange("(k p) c -> p k c", p=128))
            nc.gpsimd.dma_start(out=wv_sb[:], in_=wv_d.rearrange("(k p) c -> p k c", p=128))
            bqk_sb = [const.tile([128, 1], F32, name=f"bqk{m}", tag=f"bqk{m}")
                      for m in range(3)]
            for m in range(3):
                nc.sync.dma_start(out=bqk_sb[m][:], in_=bqk_d[128 * m:128 * (m + 1), :])
            wf01_sb = const.tile([128, EMBED_DIM], BF16)
            wf2_sb = const.tile([64, EMBED_DIM], BF16)
            nc.gpsimd.dma_start(out=wf01_sb[:], in_=wf_d[0:128, :])
            nc.gpsimd.dma_start(out=wf2_sb[:], in_=wf_d[128:192, :])
            ident_f = const.tile([128, 128], F32)
            make_identity(nc, ident_f)
            ident_b = const.tile([128, 128], BF16)
            nc.vector.tensor_copy(out=ident_b[:], in_=ident_f[:])

            # ---- persistent activations ----
            qA = persist.tile([128, T], BF16)   # q0 @0:64, q1 @64:128
            kA = persist.tile([128, T], BF16)   # k0 @0:64, k1 @64:128
            qB = persist.tile([64, T], BF16)    # q2
            kB = persist.tile([64, T], BF16)    # k2
            # [v | 1] per (key tile, head).  Double-buffered by body parity
            # (the next body's projection writes them while this body's
            # final-quarter PVs still read).
            v_alls = [persist.tile([128, NT, 3, 66], BF16, name=f"v_all{p}",
                                   tag=f"v_all{p}") for p in range(2)]
            for p in range(2):
                nc.vector.memset(v_alls[p][:], 1.0)   # col 64 stays 1.0
            ot01 = persist.tile([128, T], BF16)  # heads 0 (@0:64) & 1 (@64:128)
            ot2 = persist.tile([64, T], BF16)    # head 2

            env = locals()
            bodies = [_make_body(nc, tc, rep, env) for rep in range(repeat)]
            bodies[0][0]()
            for rep in range(repeat):
                nxt = bodies[rep + 1][0] if rep + 1 < repeat else None
                bodies[rep][1](nxt)

    nc.compile()
    return nc


def _make_body(nc, tc, rep, env):
    """Build one body's emission closures; returns (prologue, main)."""
    import os
    from concourse import mybir
    kdup = os.environ.get("KDUP", "")

    F32 = mybir.dt.float32
    F32R = mybir.dt.float32r
    BF16 = mybir.dt.bfloat16
    F16 = mybir.dt.float16
    Exp = mybir.ActivationFunctionType.Exp
    MULT = mybir.AluOpType.mult
    GE = mybir.AluOpType.is_ge
    DR = mybir.MatmulPerfMode.DoubleRow

    xT_d, out_d = env["xT_d"], env["out_d"]
    wqk_sb, wv_sb = env["wqk_sb"], env["wv_sb"]
    bqk_sb = env["bqk_sb"]
    wf01_sb, wf2_sb = env["wf01_sb"], env["wf2_sb"]
    ident_b = env["ident_b"]
    qA, kA, qB, kB = env["qA"], env["kA"], env["qB"], env["kB"]
    v_all = env["v_alls"][rep % 2]
    ot01, ot2 = env["ot01"], env["ot2"]
    dmae = [nc.sync, nc.gpsimd]

    sbp, psp = env["sbp"], env["psp"]

    # ---- input DMA: xT as 4 chunks of [128, 6, 512] ----
    xT_t = [sbp.tile([128, 6, 512], BF16, name=f"xT{rep}_{n}", tag=f"xT{n}")
            for n in range(4)]

    def emit_xt_dma():
        for n in range(4):
            for k in range(6):
                nc.sync.dma_start(
                    out=xT_t[n][:, k, :],
                    in_=xT_d[128 * k:128 * (k + 1), 512 * n:512 * (n + 1)])

    gidx = [0]

    def st_tile(shape, name):
        t = psp.tile(shape, F32, name=name, tag=f"stA{gidx[0] % 2}")
        gidx[0] += 1
        return t

    def qk_group(m, n):
        # m: 0=[q0q1]->qA, 1=[k0k1]->kA, 2=[q2|k2]->qB+kB
        c0, c1 = 128 * m, 128 * (m + 1)
        ps = st_tile([128, 512], f"pg{rep}_{m}{n}")
        for k in range(6):
            nc.tensor.matmul(ps[:], lhsT=wqk_sb[:, k, c0:c1],
                             rhs=xT_t[n][:, k, :], start=(k == 0), stop=(k == 5))
        nsl = slice(512 * n, 512 * (n + 1))
        if m < 2:
            dst = qA if m == 0 else kA
            nc.vector.tensor_scalar_add(out=dst[:, nsl], in0=ps[:],
                                        scalar1=bqk_sb[m][:])
        else:
            nc.vector.tensor_scalar_add(out=qB[:, nsl], in0=ps[0:64, :],
                                        scalar1=bqk_sb[2][0:64, :])
            nc.vector.tensor_scalar_add(out=kB[:, nsl], in0=ps[64:128, :],
                                        scalar1=bqk_sb[2][64:128, :])

    def v_group(ii, n):
        # token tile t = 4n+ii: v^T(t) = x_tile @ wv -> [128 tokens, 192]
        t = 4 * n + ii
        ps = st_tile([128, 192], f"pv{rep}_{t}")
        for k in range(6):
            nc.tensor.matmul(ps[:], lhsT=xT_t[n][:, k, 128 * ii:128 * (ii + 1)],
                             rhs=wv_sb[:, k, :], start=(k == 0), stop=(k == 5))
        nc.vector.tensor_copy(
            out=v_all[:, t, :, 0:64],
            in_=ps[:].rearrange("p (h d) -> p h d", h=3))

    def proj_chunk(n):
        for m in range(3):
            qk_group(m, n)
        for ii in range(4):
            v_group(ii, n)

    # ---- attention ----
    def KQ(h):
        if h == 0:
            return kA[0:64], qA[0:64]
        if h == 1:
            return kA[64:128], qA[64:128]
        return kB[:], qB[:]

    pts = {}    # (h, q, p) -> pt tile
    pvps = {}   # q -> [per-head psum tile [128, 4, 128] f32 (qc, dims|l)]
    pvst = {}   # (q, h) -> started flag for psum zero-region

    def diag_mask(pt, c0):
        # zero pt[:, c0:c0+128] where key row p > query col offset c
        nc.gpsimd.affine_select(
            out=pt[:, c0:c0 + 128], in_=pt[:, c0:c0 + 128],
            compare_op=GE, fill=0.0,
            base=0, channel_multiplier=-1, pattern=[[1, 128]])

    def qk_unit(h, qq, p):
        """Score tile + exp for unit p of quarter qq, head h."""
        Kh, Qh = KQ(h)
        base = 512 * qq
        nfull = 2 * qq

        def kblk(i):
            return Kh[:, 128 * i:128 * (i + 1)]

        def qrng(a, b):
            return Qh[:, base + a:base + b]

        if p < nfull:
            # full pair: key blocks 2p, 2p+1, all 512 queries
            st = st_tile([128, 1024], f"st{rep}_{h}{qq}{p}")
            if "2" in kdup and h < 2:
                # probe: same matmul on the OPPOSITE row half, spare bank
                Kf, Qf = KQ(1 - h)
                dps = psp.tile([128, 512], F32, name=f"dp{rep}_{h}{qq}{p}",
                               tag="bcp")
                nc.tensor.matmul(dps[:], lhsT=Kf[:, 128 * 2 * p:128 * (2 * p + 1)],
                                 rhs=Qf[:, 512 * qq:512 * qq + 512],
                                 start=True, stop=True, skip_group_check=True)
            for _ in range(2 if "q" in kdup else 1):
                nc.tensor.matmul(st[:, 0:512], lhsT=kblk(2 * p), rhs=qrng(0, 512),
                                 start=True, stop=True, skip_group_check=True)
                nc.tensor.matmul(st[:, 512:1024], lhsT=kblk(2 * p + 1),
                                 rhs=qrng(0, 512), start=True, stop=True,
                                 skip_group_check=True)
            pt = sbp.tile([128, 1024], BF16, name=f"pF{rep}_{h}{qq}{p}",
                          tag=f"pF{h}_{qq % 2}_{p}")
            nc.scalar.activation(out=pt[:], in_=st[:], func=Exp,
                                 scale=float(SCALE))
            if "e" in kdup:
                dmy = sbp.tile([128, 1024], BF16, name=f"dm{rep}_{h}{qq}{p}",
                               tag=f"dm{p % 2}")
                nc.scalar.activation(out=dmy[:], in_=st[:], func=Exp,
                                     scale=float(SCALE))
            pts[(h, qq, p)] = pt
        elif p == nfull:
            # D1: bank A = [A-main 384 | A-diag 128], bank B = [B 384 | C-solo 128]
            iA = 4 * qq
            st = st_tile([128, 1024], f"sd{rep}_{h}{qq}")
            nc.tensor.matmul(st[:, 0:384], lhsT=kblk(iA), rhs=qrng(128, 512),
                             start=True, stop=False)
            nc.tensor.matmul(st[:, 384:512], lhsT=kblk(iA), rhs=qrng(0, 128),
                             start=False, stop=True)
            nc.tensor.matmul(st[:, 512:896], lhsT=kblk(iA + 1), rhs=qrng(128, 512),
                             start=True, stop=False)
            nc.tensor.matmul(st[:, 896:1024], lhsT=kblk(iA + 2), rhs=qrng(256, 384),
                             start=False, stop=True)
            pt = sbp.tile([128, 1024], BF16, name=f"pD1{rep}_{h}{qq}",
                          tag=f"pD1{h}_{qq % 2}")
            nc.scalar.activation(out=pt[:], in_=st[:], func=Exp,
                                 scale=float(SCALE))
            diag_mask(pt, 384)        # A's diagonal (queries 0:128)
            diag_mask(pt, 512)        # B's diagonal (queries 128:256)
            diag_mask(pt, 896)        # C's diagonal (queries 256:384)
            pts[(h, qq, p)] = pt
        else:
            # D2: [C-main 128 | D 128]
            iA = 4 * qq
            st = st_tile([128, 256], f"s2{rep}_{h}{qq}")
            nc.tensor.matmul(st[:, 0:128], lhsT=kblk(iA + 2), rhs=qrng(384, 512),
                             start=True, stop=False)
            nc.tensor.matmul(st[:, 128:256], lhsT=kblk(iA + 3), rhs=qrng(384, 512),
                             start=False, stop=True)
            pt = sbp.tile([128, 256], BF16, name=f"pD2{rep}_{h}{qq}",
                          tag=f"pD2{h}_{qq % 2}")
            nc.scalar.activation(out=pt[:], in_=st[:], func=Exp,
                                 scale=float(SCALE))
            diag_mask(pt, 128)        # D's diagonal (queries 384:512)
            pts[(h, qq, p)] = pt

    def pv_unit(h, pvq, p):
        """Flipped PV: pt chunks are STATIONARY ([128 keys, 128 queries]),
        v_all [128, 65] streams -> psum [128 queries, 4qc, 65] per head.
        Streams 65 cols/block-chunk instead of up-to-512: PV runs at the
        output-element bound (8,840 cycles/head vs 17,408)."""
        pvp = pvps[pvq][h]
        nfull = 2 * pvq
        pt = pts.pop((h, pvq, p))

        def mm(qc, c0, i, stop=False):
            st = not pvst.get((pvq, h), False)
            pvst[(pvq, h)] = True
            nc.tensor.matmul(pvp[:, qc, 0:65], lhsT=pt[:, c0:c0 + 128],
                             rhs=v_all[:, i, h, 0:65], start=st, stop=stop)

        if p < nfull:
            for j in range(2):
                for qc in range(4):
                    mm(qc, 512 * j + 128 * qc, 2 * p + j)
        elif p == nfull:
            iA = 4 * pvq
            for qc in (1, 2, 3):
                mm(qc, 128 * (qc - 1), iA)              # A-main
            mm(0, 384, iA)                              # A-diag
            for qc in (1, 2, 3):
                mm(qc, 512 + 128 * (qc - 1), iA + 1)    # B
            mm(2, 896, iA + 2)                          # C-solo
        else:
            iA = 4 * pvq
            mm(3, 0, iA + 2)                            # C-main
            mm(3, 128, iA + 3, stop=True)               # D

    def normalize(q):
        """l sits on the 65th column per query-partition: reciprocal +
        per-partition scale on DVE, then PE-transpose back to dim-major
        ot01/ot2 for the output projection."""
        pvp = pvps.pop(q)
        rcs = []
        for h in range(3):
            rc = sbp.tile([128, 4, 1], F32, name=f"rc{rep}_{q}{h}",
                          tag=f"rc{h}")
            with nc.allow_low_precision(reason="f32 recip"):
                nc.vector.reciprocal(out=rc[:], in_=pvp[h][:, :, 64:65])
            rcs.append(rc)
        otns = [sbp.tile([128, 3, 64], BF16, name=f"on{rep}_{4 * q + qc}",
                         tag=f"on{qc}") for qc in range(4)]
        for qc in range(4):
            for h in range(3):
                nc.vector.tensor_scalar_mul(out=otns[qc][:, h, :],
                                            in0=pvp[h][:, qc, 0:64],
                                            scalar1=rcs[h][:, qc, :])
        for qc in range(4):
            t = 4 * q + qc
            tsl = slice(128 * t, 128 * (t + 1))
            dsts = [ot01[0:64, tsl], ot01[64:128, tsl], ot2[0:64, tsl]]
            for h in range(3):
                tp = psp.tile([64, 128], BF16, name=f"tp{rep}_{t}{h}",
                              tag=f"pv{h}")
                nc.tensor.transpose(tp[:], otns[qc][:, h, :], ident_b[:])
                nc.vector.tensor_copy(out=dsts[h], in_=tp[:])

    def phase3_tile(i):
        # out[128i:128i+128, :] = [ot01; ot2][:, tile i].T @ wf
        fpa = psp.tile([128, 512], F32, name=f"fpa{rep}_{i}", tag="bcp")
        fpb = psp.tile([128, 256], F32, name=f"fpb{rep}_{i}",
                       tag=f"pv{i % 3}")
        csl = slice(128 * i, 128 * (i + 1))
        for (fp, n0, n1) in [(fpa, 0, 512), (fpb, 512, 768)]:
            nc.tensor.matmul(fp[:, 0:n1 - n0], lhsT=ot01[:, csl],
                             rhs=wf01_sb[:, n0:n1], start=True, stop=False)
            nc.tensor.matmul(fp[:, 0:n1 - n0], lhsT=ot2[:, csl],
                             rhs=wf2_sb[:, n0:n1], start=False, stop=True)
        ob = sbp.tile([128, EMBED_DIM], F16, name=f"ob{rep}_{i}",
                      tag=f"ob{i % 3}")
        nc.vector.tensor_copy(out=ob[:, 0:512], in_=fpa[:])
        nc.vector.tensor_copy(out=ob[:, 512:768], in_=fpb[:])
        dmae[i % 2].dma_start(out=out_d[128 * i:128 * (i + 1), :], in_=ob[:])

    # ---- emission closures ----
    def prologue():
        emit_xt_dma()
        proj_chunk(0)
        for p in range(2):          # quarter 0 bootstrap: D1, D2
            for h in range(3):
                qk_unit(h, 0, p)

    def main(next_prologue=None):
        # normalize(q)/phase3(q) are deferred into step q+1, AFTER its
        # projection chunk: the proj matmuls sit ahead of the recip/bcast
        # chain in the in-order PE queue, hiding the chain's DVE latency.
        pending = None
        for q in range(4):
            if q == 3 and next_prologue is not None:
                next_prologue()
            if q == 0:
                proj_chunk(1)       # feeds stream 0's QK(1)
            if pending is not None:
                normalize(pending)
                for i in range(4 * pending, 4 * pending + 4):
                    phase3_tile(i)
            qq = q + 1 if q < 3 else None
            pvps[q] = [psp.tile([128, 4, 128], F32, name=f"pv{rep}_{q}{h}",
                                tag=f"pv{h}") for h in range(3)]
            # chunk q+2 feeds stream q+1: its groups interleave into THIS
            # stream as ACT-free PE filler (the stream is exp-paced; the
            # fillers absorb the stA-rotation wait for free)
            nf = q + 2
            fillers = ([lambda m=m: qk_group(m, nf) for m in range(3)] +
                       [lambda ii=ii: v_group(ii, nf) for ii in range(4)]
                       if nf <= 3 else [])
            nqk = 2 * (q + 1) + 2 if qq is not None else 0
            npv = 2 * q + 2
            for p in range(max(nqk, npv)):
                for h in range(3):
                    if p < nqk:
                        qk_unit(h, qq, p)
                    if p < npv:
                        pv_unit(h, q, p)
                if p < len(fillers):
                    fillers[p]()
            for f in fillers[max(nqk, npv):]:
                f()
            pending = q
        normalize(3)
        for i in range(12, 16):
            phase3_tile(i)

    return prologue, main


def _prep_inputs(x, w_qkv, b_qkv, w_final):
    """Build the 8 per-core input maps from the full inputs."""
    import ml_dtypes
    fp8 = ml_dtypes.float8_e4m3
    bf16 = ml_dtypes.bfloat16

    x = np.asarray(x, dtype=np.float32)
    w_qkv = np.asarray(w_qkv, dtype=np.float32)
    b_qkv = np.asarray(b_qkv, dtype=np.float32)
    w_final = np.asarray(w_final, dtype=np.float32)
    E = EMBED_DIM

    in_maps = []
    for c in range(N_CORES):
        b = c // 4
        g = c % 4
        heads = [3 * g, 3 * g + 1, 3 * g + 2]
        hr = [np.arange(64 * h, 64 * h + 64) for h in heads]
        # [q0 q1 | k0 k1 | q2 | k2]
        rows_qk = np.concatenate([hr[0], hr[1], E + hr[0], E + hr[1], hr[2], E + hr[2]])
        rows_v = np.concatenate(hr) + 2 * E
        xT = np.ascontiguousarray(x[b].T).astype(bf16)               # [768, 2048]
        wqk = np.ascontiguousarray(w_qkv[rows_qk].T).astype(bf16)    # [768, 384]
        wv = np.ascontiguousarray(w_qkv[rows_v].T).astype(bf16)      # [768, 192]
        bqk = np.ascontiguousarray(b_qkv[rows_qk][:, None])
        wf = np.ascontiguousarray(w_final[:, np.concatenate(hr)].T).astype(bf16)
        in_maps.append({"xT": xT, "wqk": wqk, "wv": wv, "bqk": bqk, "wf": wf})
    return in_maps


def kernel(x, w_qkv, b_qkv, w_final, _trace=False):
    from concourse.bass_utils import run_bass_kernel_spmd

    if "nc" not in _state:
        _state["nc"] = _build()
    nc = _state["nc"]

    in_maps = _prep_inputs(x, w_qkv, b_qkv, w_final)
    res = run_bass_kernel_spmd(nc, in_maps, list(range(N_CORES)), trace=_trace)
    _state["last_result"] = res

    w_final = np.asarray(w_final, dtype=np.float64)
    b_qkv = np.asarray(b_qkv, dtype=np.float64)
    # v bias folds into a constant row: softmax weights sum to 1
    const_row = w_final @ b_qkv[2 * EMBED_DIM:]

    out = np.empty((B, T, EMBED_DIM), dtype=np.float32)
    for b in range(B):
        acc = np.zeros((T, EMBED_DIM), dtype=np.float64)
        for g in range(4):
            acc += res.results[4 * b + g]["out_p"].astype(np.float64)
        out[b] = (acc + const_row).astype(np.float32)
    return out


# revision 27
# speedup vs baseline: 1.0948x; 1.0108x over previous
"""Multi-head causal attention (B=2, T=2048, E=768, H=12, D=64) on 8 trn2 cores.

Sharding: core c handles batch b=c//4 and heads [3g, 3g+1, 3g+2] (g=c%4).
Each core computes its 3 heads' attention plus their partial contribution to
the final projection; the host sums the 4 partials per batch.

Differences vs the original transpose-based kernel:
- V is projected directly in [tokens, dims] orientation (lhsT = x token
  tile), eliminating all 48 PE transposes (-12k PE cycles/body); its bias is
  folded into a host-side constant row (softmax weights sum to 1).
- Unified per-head attention over key-block PAIRS, bank-aligned: full pair
  tiles are [block 2p | block 2p+1] (1024 cols, ONE exp); the 4 diagonal
  blocks A..D of each quarter pack into two tiles:
    st1 (2 banks): [A-main 384 | A-diag 128 | B 384 | C-solo 128]
    st2 (1 bank):  [C-main 128 | D 128]
  Every matmul output stays inside a 2KB psum bank (hard hw rule) and each
  tile needs one exp (60 exps/body).  Diagonal 128-blocks are masked AFTER
  exp by gpsimd affine_select on the SBUF pt tiles.
- FLIPPED PV: the exp'd score chunks ([128 keys, 128 queries]) are the
  STATIONARY operand and v_all [128, 65] streams, accumulating O^T per
  query-chunk into psum [128 queries, 4, 65|l].  PV streams 65 cols per
  (block, chunk) = the output-element bound: 8,840 cycles/head vs 17,408
  for the classic orientation (which is moving-column-bound on pt).
  Bonus: l lands on the PARTITION axis, so normalize is a plain DVE
  reciprocal + per-partition tensor_scalar (the whole PE-broadcast /
  copy machinery is gone); PE transposes (16 tiles x 3 heads) restore
  dim-major ot01/ot2 for the output projection.
- normalize(q) + phase3(q) are deferred into step q+1 AFTER its projection
  chunk, so their dependency chain hides behind ~6k cycles of independent
  projection matmuls in the in-order PE queue.
- Output partials are DMA'd as fp16 (halves output HBM traffic; summed in
  f64 on the host).

Everything numeric is bf16 into f32 psum.  fp8(e4m3) + DoubleRow was built
and measured: it passes BIR/hardware fine (see git-less probe history) but
any fp8 tensor in the q/k path, v, or even just the output projection blows
the rel-2e-2 max-norm budget (exp amplifies score jitter; peaked softmax rows
expose raw v quantization; max over 3.1M outputs sits ~5 sigma up), so it is
not used.

Measured: ~118us/body (vs 171us for the classic-orientation kernel).
PE-cycle-bound; ~165k PE cycles/body (proj 55k, QK 52k, PV 26.5k,
transposes 6k, out-proj 25k).  KDUP=q probes measured ~1.13GHz marginal in
the OLD long-stream regime, yet this kernel beats the 1.2GHz pure-stream
bound -- the clock gate rewards the flipped PV's short clean streams, so
treat clock models as advisory and measure.  ACT (exp, 55us) and DVE have
slack.  Row-tile co-execution does NOT exist (probe_rows.py: packed/serial
= 0.73, just overhead hiding), and fp8/DoubleRow is numerically dead at
this tolerance everywhere.

PSUM banks (8): stA0/stA1 [128,1024] f32 (2 each) + otl0/1/2 [128,512] + bcp.
`repeat` unrolls the body N times in one NEFF; test.py measures per-body HW
time as the slope of wall time vs N.  KDUP=q/e/2 add duplicate work for
differential load probing (default off).
"""
import numpy as np

EMBED_DIM = 768
B = 2
T = 2048
N_CORES = 8
NT = T // 128           # 16 key/query tiles
SCALE = 1.0 / np.sqrt(64.0)

_state = {}


def _build(repeat=1):
    import concourse.tile as tile
    from concourse import bacc, mybir
    from concourse.masks import make_identity

    F32 = mybir.dt.float32
    F32R = mybir.dt.float32r
    BF16 = mybir.dt.bfloat16
    FP8 = mybir.dt.float8e4
    F16 = mybir.dt.float16

    nc = bacc.Bacc("TRN2", target_bir_lowering=False, debug=False)

    xT_d = nc.dram_tensor("xT", [EMBED_DIM, T], BF16, kind="ExternalInput").ap()
    # columns ordered [q0 q1 | k0 k1 | q2 | k2]
    wqk_d = nc.dram_tensor("wqk", [EMBED_DIM, 384], BF16, kind="ExternalInput").ap()
    wv_d = nc.dram_tensor("wv", [EMBED_DIM, 192], BF16, kind="ExternalInput").ap()
    bqk_d = nc.dram_tensor("bqk", [384, 1], F32, kind="ExternalInput").ap()
    # w_final^T rows for this core's 192 dims: [0:128) and [128:192)
    wf_d = nc.dram_tensor("wf", [192, EMBED_DIM], BF16, kind="ExternalInput").ap()
    out_d = nc.dram_tensor("out_p", [T, EMBED_DIM], F16, kind="ExternalOutput").ap()

    with tile.TileContext(nc) as tc:
        with tc.tile_pool(name="const", bufs=1) as const, \
             tc.tile_pool(name="persist", bufs=1) as persist, \
             tc.tile_pool(name="sbod", bufs=1) as sbp, \
             tc.tile_pool(name="psod", bufs=1, space="PSUM") as psp:
            # ---- constants ----
            wqk_sb = const.tile([128, 6, 384], BF16)
            wv_sb = const.tile([128, 6, 192], BF16)
            nc.sync.dma_start(out=wqk_sb[:], in_=wqk_d.rearrange("(k p) c -> p k c", p=128))
            nc.gpsimd.dma_start(out=wv_sb[:], in_=wv_d.rearrange("(k p) c -> p k c", p=128))
            bqk_sb = [const.tile([128, 1], F32, name=f"bqk{m}", tag=f"bqk{m}")
                      for m in range(3)]
            for m in range(3):
                nc.sync.dma_start(out=bqk_sb[m][:], in_=bqk_d[128 * m:128 * (m + 1), :])
            wf01_sb = const.tile([128, EMBED_DIM], BF16)
            wf2_sb = const.tile([64, EMBED_DIM], BF16)
            nc.gpsimd.dma_start(out=wf01_sb[:], in_=wf_d[0:128, :])
            nc.gpsimd.dma_start(out=wf2_sb[:], in_=wf_d[128:192, :])
            ident_f = const.tile([128, 128], F32)
            make_identity(nc, ident_f)
            ident_b = const.tile([128, 128], BF16)
            nc.vector.tensor_copy(out=ident_b[:], in_=ident_f[:])

            # ---- persistent activations ----
            qA = persist.tile([128, T], BF16)   # q0 @0:64, q1 @64:128
            kA = persist.tile([128, T], BF16)   # k0 @0:64, k1 @64:128
            qB = persist.tile([64, T], BF16)    # q2
            kB = persist.tile([64, T], BF16)    # k2
            # [v | 1] per (key tile, head).  Double-buffered by body parity
            # (the next body's projection writes them while this body's
            # final-quarter PVs still read).
            v_alls = [persist.tile([128, NT, 3, 66], BF16, name=f"v_all{p}",
                                   tag=f"v_all{p}") for p in range(2)]
            for p in range(2):
                nc.vector.memset(v_alls[p][:], 1.0)   # col 64 stays 1.0
            ot01 = persist.tile([128, T], BF16)  # heads 0 (@0:64) & 1 (@64:128)
            ot2 = persist.tile([64, T], BF16)    # head 2

            env = locals()
            bodies = [_make_body(nc, tc, rep, env) for rep in range(repeat)]
            bodies[0][0]()
            for rep in range(repeat):
                nxt = bodies[rep + 1][0] if rep + 1 < repeat else None
                bodies[rep][1](nxt)

    nc.compile()
    return nc


def _make_body(nc, tc, rep, env):
    """Build one body's emission closures; returns (prologue, main)."""
    import os
    from concourse import mybir
    kdup = os.environ.get("KDUP", "")

    F32 = mybir.dt.float32
    F32R = mybir.dt.float32r
    BF16 = mybir.dt.bfloat16
    F16 = mybir.dt.float16
    Exp = mybir.ActivationFunctionType.Exp
    MULT = mybir.AluOpType.mult
    GE = mybir.AluOpType.is_ge
    DR = mybir.MatmulPerfMode.DoubleRow

    xT_d, out_d = env["xT_d"], env["out_d"]
    wqk_sb, wv_sb = env["wqk_sb"], env["wv_sb"]
    bqk_sb = env["bqk_sb"]
    wf01_sb, wf2_sb = env["wf01_sb"], env["wf2_sb"]
    ident_b = env["ident_b"]
    qA, kA, qB, kB = env["qA"], env["kA"], env["qB"], env["kB"]
    v_all = env["v_alls"][rep % 2]
    ot01, ot2 = env["ot01"], env["ot2"]
    dmae = [nc.sync, nc.gpsimd]

    sbp, psp = env["sbp"], env["psp"]

    # ---- input DMA: xT as 4 chunks of [128, 6, 512] ----
    xT_t = [sbp.tile([128, 6, 512], BF16, name=f"xT{rep}_{n}", tag=f"xT{n}")
            for n in range(4)]

    def emit_xt_dma():
        for n in range(4):
            for k in range(6):
                nc.sync.dma_start(
                    out=xT_t[n][:, k, :],
                    in_=xT_d[128 * k:128 * (k + 1), 512 * n:512 * (n + 1)])

    gidx = [0]

    def st_tile(shape, name):
        t = psp.tile(shape, F32, name=name, tag=f"stA{gidx[0] % 2}")
        gidx[0] += 1
        return t

    def qk_group(m, n):
        # m: 0=[q0q1]->qA, 1=[k0k1]->kA, 2=[q2|k2]->qB+kB
        c0, c1 = 128 * m, 128 * (m + 1)
        ps = st_tile([128, 512], f"pg{rep}_{m}{n}")
        for k in range(6):
            nc.tensor.matmul(ps[:], lhsT=wqk_sb[:, k, c0:c1],
                             rhs=xT_t[n][:, k, :], start=(k == 0), stop=(k == 5))
        nsl = slice(512 * n, 512 * (n + 1))
        if m < 2:
            dst = qA if m == 0 else kA
            nc.vector.tensor_scalar_add(out=dst[:, nsl], in0=ps[:],
                                        scalar1=bqk_sb[m][:])
        else:
            nc.vector.tensor_scalar_add(out=qB[:, nsl], in0=ps[0:64, :],
                                        scalar1=bqk_sb[2][0:64, :])
            nc.vector.tensor_scalar_add(out=kB[:, nsl], in0=ps[64:128, :],
                                        scalar1=bqk_sb[2][64:128, :])

    def v_group(ii, n):
        # token tile t = 4n+ii: v^T(t) = x_tile @ wv -> [128 tokens, 192]
        t = 4 * n + ii
        ps = st_tile([128, 192], f"pv{rep}_{t}")
        for k in range(6):
            nc.tensor.matmul(ps[:], lhsT=xT_t[n][:, k, 128 * ii:128 * (ii + 1)],
                             rhs=wv_sb[:, k, :], start=(k == 0), stop=(k == 5))
        nc.vector.tensor_copy(
            out=v_all[:, t, :, 0:64],
            in_=ps[:].rearrange("p (h d) -> p h d", h=3))

    def proj_chunk(n):
        for m in range(3):
            qk_group(m, n)
        for ii in range(4):
            v_group(ii, n)

    # ---- attention ----
    def KQ(h):
        if h == 0:
            return kA[0:64], qA[0:64]
        if h == 1:
            return kA[64:128], qA[64:128]
        return kB[:], qB[:]

    pts = {}    # (h, q, p) -> pt tile
    pvps = {}   # q -> [per-head psum tile [128, 4, 128] f32 (qc, dims|l)]
    pvst = {}   # (q, h) -> started flag for psum zero-region

    def diag_mask(pt, c0):
        # zero pt[:, c0:c0+128] where key row p > query col offset c
        nc.gpsimd.affine_select(
            out=pt[:, c0:c0 + 128], in_=pt[:, c0:c0 + 128],
            compare_op=GE, fill=0.0,
            base=0, channel_multiplier=-1, pattern=[[1, 128]])

    def qk_unit(h, qq, p):
        """Score tile + exp for unit p of quarter qq, head h."""
        Kh, Qh = KQ(h)
        base = 512 * qq
        nfull = 2 * qq

        def kblk(i):
            return Kh[:, 128 * i:128 * (i + 1)]

        def qrng(a, b):
            return Qh[:, base + a:base + b]

        if p < nfull:
            # full pair: key blocks 2p, 2p+1, all 512 queries
            st = st_tile([128, 1024], f"st{rep}_{h}{qq}{p}")
            if "2" in kdup and h < 2:
                # probe: same matmul on the OPPOSITE row half, spare bank
                Kf, Qf = KQ(1 - h)
                dps = psp.tile([128, 512], F32, name=f"dp{rep}_{h}{qq}{p}",
                               tag="bcp")
                nc.tensor.matmul(dps[:], lhsT=Kf[:, 128 * 2 * p:128 * (2 * p + 1)],
                                 rhs=Qf[:, 512 * qq:512 * qq + 512],
                                 start=True, stop=True, skip_group_check=True)
            for _ in range(2 if "q" in kdup else 1):
                nc.tensor.matmul(st[:, 0:512], lhsT=kblk(2 * p), rhs=qrng(0, 512),
                                 start=True, stop=True, skip_group_check=True)
                nc.tensor.matmul(st[:, 512:1024], lhsT=kblk(2 * p + 1),
                                 rhs=qrng(0, 512), start=True, stop=True,
                                 skip_group_check=True)
            pt = sbp.tile([128, 1024], BF16, name=f"pF{rep}_{h}{qq}{p}",
                          tag=f"pF{h}_{qq % 2}_{p}")
            nc.scalar.activation(out=pt[:], in_=st[:], func=Exp,
                                 scale=float(SCALE))
            if "e" in kdup:
                dmy = sbp.tile([128, 1024], BF16, name=f"dm{rep}_{h}{qq}{p}",
                               tag=f"dm{p % 2}")
                nc.scalar.activation(out=dmy[:], in_=st[:], func=Exp,
                                     scale=float(SCALE))
            pts[(h, qq, p)] = pt
        elif p == nfull:
            # D1: bank A = [A-main 384 | A-diag 128], bank B = [B 384 | C-solo 128]
            iA = 4 * qq
            st = st_tile([128, 1024], f"sd{rep}_{h}{qq}")
            nc.tensor.matmul(st[:, 0:384], lhsT=kblk(iA), rhs=qrng(128, 512),
                             start=True, stop=False)
            nc.tensor.matmul(st[:, 384:512], lhsT=kblk(iA), rhs=qrng(0, 128),
                             start=False, stop=True)
            nc.tensor.matmul(st[:, 512:896], lhsT=kblk(iA + 1), rhs=qrng(128, 512),
                             start=True, stop=False)
            nc.tensor.matmul(st[:, 896:1024], lhsT=kblk(iA + 2), rhs=qrng(256, 384),
                             start=False, stop=True)
            pt = sbp.tile([128, 1024], BF16, name=f"pD1{rep}_{h}{qq}",
                          tag=f"pD1{h}_{qq % 2}")
            nc.scalar.activation(out=pt[:], in_=st[:], func=Exp,
                                 scale=float(SCALE))
            diag_mask(pt, 384)        # A's diagonal (queries 0:128)
            diag_mask(pt, 512)        # B's diagonal (queries 128:256)
            diag_mask(pt, 896)        # C's diagonal (queries 256:384)
            pts[(h, qq, p)] = pt
        else:
            # D2: [C-main 128 | D 128]
            iA = 4 * qq
            st = st_tile([128, 256], f"s2{rep}_{h}{qq}")
            nc.tensor.matmul(st[:, 0:128], lhsT=kblk(iA + 2), rhs=qrng(384, 512),
                             start=True, stop=False)
            nc.tensor.matmul(st[:, 128:256], lhsT=kblk(iA + 3), rhs=qrng(384, 512),
                             start=False, stop=True)
            pt = sbp.tile([128, 256], BF16, name=f"pD2{rep}_{h}{qq}",
                          tag=f"pD2{h}_{qq % 2}")
            nc.scalar.activation(out=pt[:], in_=st[:], func=Exp,
                                 scale=float(SCALE))
            diag_mask(pt, 128)        # D's diagonal (queries 384:512)
            pts[(h, qq, p)] = pt

    def pv_unit(h, pvq, p):
        """Flipped PV: pt chunks are STATIONARY ([128 keys, 128 queries]),
        v_all [128, 65] streams -> psum [128 queries, 4qc, 65] per head.
        Streams 65 cols/block-chunk instead of up-to-512: PV runs at the
        output-element bound (8,840 cycles/head vs 17,408)."""
        pvp = pvps[pvq][h]
        nfull = 2 * pvq
        pt = pts.pop((h, pvq, p))

        def mm(qc, c0, i, stop=False):
            st = not pvst.get((pvq, h), False)
            pvst[(pvq, h)] = True
            nc.tensor.matmul(pvp[:, qc, 0:65], lhsT=pt[:, c0:c0 + 128],
                             rhs=v_all[:, i, h, 0:65], start=st, stop=stop)

        if p < nfull:
            for j in range(2):
                for qc in range(4):
                    mm(qc, 512 * j + 128 * qc, 2 * p + j)
        elif p == nfull:
            iA = 4 * pvq
            for qc in (1, 2, 3):
                mm(qc, 128 * (qc - 1), iA)              # A-main
            mm(0, 384, iA)                              # A-diag
            for qc in (1, 2, 3):
                mm(qc, 512 + 128 * (qc - 1), iA + 1)    # B
            mm(2, 896, iA + 2)                          # C-solo
        else:
            iA = 4 * pvq
            mm(3, 0, iA + 2)                            # C-main
            mm(3, 128, iA + 3, stop=True)               # D

    def normalize(q):
        """l sits on the 65th column per query-partition: reciprocal +
        per-partition scale on DVE, then PE-transpose back to dim-major
        ot01/ot2 for the output projection."""
        pvp = pvps.pop(q)
        rcs = []
        for h in range(3):
            rc = sbp.tile([128, 4, 1], F32, name=f"rc{rep}_{q}{h}",
                          tag=f"rc{h}")
            with nc.allow_low_precision(reason="f32 recip"):
                nc.vector.reciprocal(out=rc[:], in_=pvp[h][:, :, 64:65])
            rcs.append(rc)
        otns = [sbp.tile([128, 3, 64], BF16, name=f"on{rep}_{4 * q + qc}",
                         tag=f"on{qc}") for qc in range(4)]
        for qc in range(4):
            for h in range(3):
                nc.vector.tensor_scalar_mul(out=otns[qc][:, h, :],
                                            in0=pvp[h][:, qc, 0:64],
                                            scalar1=rcs[h][:, qc, :])
        for qc in range(4):
            t = 4 * q + qc
            tsl = slice(128 * t, 128 * (t + 1))
            dsts = [ot01[0:64, tsl], ot01[64:128, tsl], ot2[0:64, tsl]]
            for h in range(3):
                tp = psp.tile([64, 128], BF16, name=f"tp{rep}_{t}{h}",
                              tag=f"pv{h}")
                nc.tensor.transpose(tp[:], otns[qc][:, h, :], ident_b[:])
                nc.vector.tensor_copy(out=dsts[h], in_=tp[:])

    def phase3_tile(i):
        # out[128i:128i+128, :] = [ot01; ot2][:, tile i].T @ wf
        fpa = psp.tile([128, 512], F32, name=f"fpa{rep}_{i}", tag="bcp")
        fpb = psp.tile([128, 256], F32, name=f"fpb{rep}_{i}",
                       tag=f"pv{i % 3}")
        csl = slice(128 * i, 128 * (i + 1))
        for (fp, n0, n1) in [(fpa, 0, 512), (fpb, 512, 768)]:
            nc.tensor.matmul(fp[:, 0:n1 - n0], lhsT=ot01[:, csl],
                             rhs=wf01_sb[:, n0:n1], start=True, stop=False)
            nc.tensor.matmul(fp[:, 0:n1 - n0], lhsT=ot2[:, csl],
                             rhs=wf2_sb[:, n0:n1], start=False, stop=True)
        ob = sbp.tile([128, EMBED_DIM], F16, name=f"ob{rep}_{i}",
                      tag=f"ob{i % 3}")
        nc.vector.tensor_copy(out=ob[:, 0:512], in_=fpa[:])
        nc.vector.tensor_copy(out=ob[:, 512:768], in_=fpb[:])
        dmae[i % 2].dma_start(out=out_d[128 * i:128 * (i + 1), :], in_=ob[:])

    # ---- emission closures ----
    def prologue():
        emit_xt_dma()
        proj_chunk(0)
        for p in range(2):          # quarter 0 bootstrap: D1, D2
            for h in range(3):
                qk_unit(h, 0, p)

    def main(next_prologue=None):
        # normalize(q)/phase3(q) are deferred into step q+1, AFTER its
        # projection chunk: the proj matmuls sit ahead of the recip/bcast
        # chain in the in-order PE queue, hiding the chain's DVE latency.
        pending = None
        for q in range(4):
            if q == 3 and next_prologue is not None:
                next_prologue()
            if q < 3:
                proj_chunk(q + 1)
            if pending is not None:
                normalize(pending)
                for i in range(4 * pending, 4 * pending + 4):
                    phase3_tile(i)
            qq = q + 1 if q < 3 else None
            pvps[q] = [psp.tile([128, 4, 128], F32, name=f"pv{rep}_{q}{h}",
                                tag=f"pv{h}") for h in range(3)]
            nqk = 2 * (q + 1) + 2 if qq is not None else 0
            npv = 2 * q + 2
            for p in range(max(nqk, npv)):
                for h in range(3):
                    if p < nqk:
                        qk_unit(h, qq, p)
                    if p < npv:
                        pv_unit(h, q, p)
            pending = q
        normalize(3)
        for i in range(12, 16):
            phase3_tile(i)

    return prologue, main


def _prep_inputs(x, w_qkv, b_qkv, w_final):
    """Build the 8 per-core input maps from the full inputs."""
    import ml_dtypes
    fp8 = ml_dtypes.float8_e4m3
    bf16 = ml_dtypes.bfloat16

    x = np.asarray(x, dtype=np.float32)
    w_qkv = np.asarray(w_qkv, dtype=np.float32)
    b_qkv = np.asarray(b_qkv, dtype=np.float32)
    w_final = np.asarray(w_final, dtype=np.float32)
    E = EMBED_DIM

    in_maps = []
    for c in range(N_CORES):
        b = c // 4
        g = c % 4
        heads = [3 * g, 3 * g + 1, 3 * g + 2]
        hr = [np.arange(64 * h, 64 * h + 64) for h in heads]
        # [q0 q1 | k0 k1 | q2 | k2]
        rows_qk = np.concatenate([hr[0], hr[1], E + hr[0], E + hr[1], hr[2], E + hr[2]])
        rows_v = np.concatenate(hr) + 2 * E
        xT = np.ascontiguousarray(x[b].T).astype(bf16)               # [768, 2048]
        wqk = np.ascontiguousarray(w_qkv[rows_qk].T).astype(bf16)    # [768, 384]
        wv = np.ascontiguousarray(w_qkv[rows_v].T).astype(bf16)      # [768, 192]
        bqk = np.ascontiguousarray(b_qkv[rows_qk][:, None])
        wf = np.ascontiguousarray(w_final[:, np.concatenate(hr)].T).astype(bf16)
        in_maps.append({"xT": xT, "wqk": wqk, "wv": wv, "bqk": bqk, "wf": wf})
    return in_maps


def kernel(x, w_qkv, b_qkv, w_final, _trace=False):
    from concourse.bass_utils import run_bass_kernel_spmd

    if "nc" not in _state:
        _state["nc"] = _build()
    nc = _state["nc"]

    in_maps = _prep_inputs(x, w_qkv, b_qkv, w_final)
    res = run_bass_kernel_spmd(nc, in_maps, list(range(N_CORES)), trace=_trace)
    _state["last_result"] = res

    w_final = np.asarray(w_final, dtype=np.float64)
    b_qkv = np.asarray(b_qkv, dtype=np.float64)
    # v bias folds into a constant row: softmax weights sum to 1
    const_row = w_final @ b_qkv[2 * EMBED_DIM:]

    out = np.empty((B, T, EMBED_DIM), dtype=np.float32)
    for b in range(B):
        acc = np.zeros((T, EMBED_DIM), dtype=np.float64)
        for g in range(4):
            acc += res.results[4 * b + g]["out_p"].astype(np.float64)
        out[b] = (acc + const_row).astype(np.float32)
    return out


# revision 29
# speedup vs baseline: 1.1924x; 1.0891x over previous
"""Multi-head causal attention (B=2, T=2048, E=768, H=12, D=64) on 8 trn2 cores.

Sharding: core c handles batch b=c//4 and heads [3g, 3g+1, 3g+2] (g=c%4).
Each core computes its 3 heads' attention plus their partial contribution to
the final projection; the host sums the 4 partials per batch.

Differences vs the original transpose-based kernel:
- V is projected directly in [tokens, dims] orientation (lhsT = x token
  tile), eliminating all 48 PE transposes (-12k PE cycles/body); its bias is
  folded into a host-side constant row (softmax weights sum to 1).
- Unified per-head attention over key-block PAIRS, bank-aligned: full pair
  tiles are [block 2p | block 2p+1] (1024 cols, ONE exp); the 4 diagonal
  blocks A..D of each quarter pack into two tiles:
    st1 (2 banks): [A-main 384 | A-diag 128 | B 384 | C-solo 128]
    st2 (1 bank):  [C-main 128 | D 128]
  Every matmul output stays inside a 2KB psum bank (hard hw rule) and each
  tile needs one exp (60 exps/body).  Diagonal 128-blocks are masked AFTER
  exp by gpsimd affine_select on the SBUF pt tiles.
- FLIPPED PV: the exp'd score chunks ([128 keys, 128 queries]) are the
  STATIONARY operand and v_all [128, 65] streams, accumulating O^T per
  query-chunk into psum [128 queries, 4, 65|l].  PV streams 65 cols per
  (block, chunk) = the output-element bound: 8,840 cycles/head vs 17,408
  for the classic orientation (which is moving-column-bound on pt).
  Bonus: l lands on the PARTITION axis, so normalize is a plain DVE
  reciprocal + per-partition tensor_scalar (the whole PE-broadcast /
  copy machinery is gone); PE transposes (16 tiles x 3 heads) restore
  dim-major ot01/ot2 for the output projection.
- normalize(q) + phase3(q) are deferred into step q+1 AFTER its projection
  chunk, so their dependency chain hides behind ~6k cycles of independent
  projection matmuls in the in-order PE queue.
- Output partials are DMA'd as fp16 (halves output HBM traffic; summed in
  f64 on the host).

Everything numeric is bf16 into f32 psum.  fp8(e4m3) + DoubleRow was built
and measured: it passes BIR/hardware fine (see git-less probe history) but
any fp8 tensor in the q/k path, v, or even just the output projection blows
the rel-2e-2 max-norm budget (exp amplifies score jitter; peaked softmax rows
expose raw v quantization; max over 3.1M outputs sits ~5 sigma up), so it is
not used.

Measured: ~118us/body (vs 171us for the classic-orientation kernel).
PE-cycle-bound; ~165k PE cycles/body (proj 55k, QK 52k, PV 26.5k,
transposes 6k, out-proj 25k).  KDUP=q probes measured ~1.13GHz marginal in
the OLD long-stream regime, yet this kernel beats the 1.2GHz pure-stream
bound -- the clock gate rewards the flipped PV's short clean streams, so
treat clock models as advisory and measure.  ACT (exp, 55us) and DVE have
slack.  Row-tile co-execution does NOT exist (probe_rows.py: packed/serial
= 0.73, just overhead hiding), and fp8/DoubleRow is numerically dead at
this tolerance everywhere.

PSUM banks (8): stA0/stA1 [128,1024] f32 (2 each) + otl0/1/2 [128,512] + bcp.
`repeat` unrolls the body N times in one NEFF; test.py measures per-body HW
time as the slope of wall time vs N.  KDUP=q/e/2 add duplicate work for
differential load probing (default off).
"""
import numpy as np

EMBED_DIM = 768
B = 2
T = 2048
N_CORES = 8
NT = T // 128           # 16 key/query tiles
SCALE = 1.0 / np.sqrt(64.0)

_state = {}


def _build(repeat=1):
    import concourse.tile as tile
    from concourse import bacc, mybir
    from concourse.masks import make_identity

    F32 = mybir.dt.float32
    F32R = mybir.dt.float32r
    BF16 = mybir.dt.bfloat16
    FP8 = mybir.dt.float8e4
    F16 = mybir.dt.float16

    nc = bacc.Bacc("TRN2", target_bir_lowering=False, debug=False)

    xT_d = nc.dram_tensor("xT", [EMBED_DIM, T], BF16, kind="ExternalInput").ap()
    # columns ordered [q0 q1 | k0 k1 | q2 | k2]
    wqk_d = nc.dram_tensor("wqk", [EMBED_DIM, 384], BF16, kind="ExternalInput").ap()
    wv_d = nc.dram_tensor("wv", [EMBED_DIM, 192], BF16, kind="ExternalInput").ap()
    bqk_d = nc.dram_tensor("bqk", [384, 1], F32, kind="ExternalInput").ap()
    # w_final^T rows for this core's 192 dims: [0:128) and [128:192)
    wf_d = nc.dram_tensor("wf", [192, EMBED_DIM], BF16, kind="ExternalInput").ap()
    out_d = nc.dram_tensor("out_p", [T, EMBED_DIM], F16, kind="ExternalOutput").ap()

    with tile.TileContext(nc) as tc:
        with tc.tile_pool(name="const", bufs=1) as const, \
             tc.tile_pool(name="persist", bufs=1) as persist, \
             tc.tile_pool(name="sbod", bufs=1) as sbp, \
             tc.tile_pool(name="psod", bufs=1, space="PSUM") as psp:
            # ---- constants ----
            wqk_sb = const.tile([128, 6, 384], BF16)
            wv_sb = const.tile([128, 6, 192], BF16)
            nc.sync.dma_start(out=wqk_sb[:], in_=wqk_d.rearrange("(k p) c -> p k c", p=128))
            nc.gpsimd.dma_start(out=wv_sb[:], in_=wv_d.rearrange("(k p) c -> p k c", p=128))
            bqk_sb = [const.tile([128, 1], F32, name=f"bqk{m}", tag=f"bqk{m}")
                      for m in range(3)]
            for m in range(3):
                nc.sync.dma_start(out=bqk_sb[m][:], in_=bqk_d[128 * m:128 * (m + 1), :])
            wf01_sb = const.tile([128, EMBED_DIM], BF16)
            wf2_sb = const.tile([64, EMBED_DIM], BF16)
            nc.gpsimd.dma_start(out=wf01_sb[:], in_=wf_d[0:128, :])
            nc.gpsimd.dma_start(out=wf2_sb[:], in_=wf_d[128:192, :])
            ident_f = const.tile([128, 128], F32)
            make_identity(nc, ident_f)
            ident_b = const.tile([128, 128], BF16)
            nc.vector.tensor_copy(out=ident_b[:], in_=ident_f[:])

            # ---- persistent activations ----
            qA = persist.tile([128, T], BF16)   # q0 @0:64, q1 @64:128
            kA = persist.tile([128, T], BF16)   # k0 @0:64, k1 @64:128
            qB = persist.tile([64, T], BF16)    # q2
            kB = persist.tile([64, T], BF16)    # k2
            # [v | 1] per (key tile, head).  Double-buffered by body parity
            # (the next body's projection writes them while this body's
            # final-quarter PVs still read).
            v_alls = [persist.tile([128, NT, 3, 66], BF16, name=f"v_all{p}",
                                   tag=f"v_all{p}") for p in range(2)]
            for p in range(2):
                nc.vector.memset(v_alls[p][:], 1.0)   # col 64 stays 1.0
            ot01 = persist.tile([128, T], BF16)  # heads 0 (@0:64) & 1 (@64:128)
            ot2 = persist.tile([64, T], BF16)    # head 2

            env = locals()
            bodies = [_make_body(nc, tc, rep, env) for rep in range(repeat)]
            bodies[0][0]()
            for rep in range(repeat):
                nxt = bodies[rep + 1][0] if rep + 1 < repeat else None
                bodies[rep][1](nxt)

    nc.compile()
    return nc


def _make_body(nc, tc, rep, env):
    """Build one body's emission closures; returns (prologue, main)."""
    import os
    from concourse import mybir
    kdup = os.environ.get("KDUP", "")

    F32 = mybir.dt.float32
    F32R = mybir.dt.float32r
    BF16 = mybir.dt.bfloat16
    F16 = mybir.dt.float16
    Exp = mybir.ActivationFunctionType.Exp
    MULT = mybir.AluOpType.mult
    GE = mybir.AluOpType.is_ge
    DR = mybir.MatmulPerfMode.DoubleRow

    xT_d, out_d = env["xT_d"], env["out_d"]
    wqk_sb, wv_sb = env["wqk_sb"], env["wv_sb"]
    bqk_sb = env["bqk_sb"]
    wf01_sb, wf2_sb = env["wf01_sb"], env["wf2_sb"]
    ident_b = env["ident_b"]
    qA, kA, qB, kB = env["qA"], env["kA"], env["qB"], env["kB"]
    v_all = env["v_alls"][rep % 2]
    ot01, ot2 = env["ot01"], env["ot2"]
    dmae = [nc.sync, nc.gpsimd]

    sbp, psp = env["sbp"], env["psp"]

    # ---- input DMA: xT as 4 chunks of [128, 6, 512] ----
    xT_t = [sbp.tile([128, 6, 512], BF16, name=f"xT{rep}_{n}", tag=f"xT{n}")
            for n in range(4)]

    def emit_xt_dma():
        for n in range(4):
            for k in range(6):
                nc.sync.dma_start(
                    out=xT_t[n][:, k, :],
                    in_=xT_d[128 * k:128 * (k + 1), 512 * n:512 * (n + 1)])

    gidx = [0]

    def st_tile(shape, name):
        t = psp.tile(shape, F32, name=name, tag=f"stA{gidx[0] % 2}")
        gidx[0] += 1
        return t

    def qk_group(m, n):
        # m: 0=[q0q1]->qA, 1=[k0k1]->kA, 2=[q2|k2]->qB+kB
        c0, c1 = 128 * m, 128 * (m + 1)
        ps = st_tile([128, 512], f"pg{rep}_{m}{n}")
        for k in range(6):
            nc.tensor.matmul(ps[:], lhsT=wqk_sb[:, k, c0:c1],
                             rhs=xT_t[n][:, k, :], start=(k == 0), stop=(k == 5))
        nsl = slice(512 * n, 512 * (n + 1))
        if m < 2:
            dst = qA if m == 0 else kA
            nc.vector.tensor_scalar_add(out=dst[:, nsl], in0=ps[:],
                                        scalar1=bqk_sb[m][:])
        else:
            nc.vector.tensor_scalar_add(out=qB[:, nsl], in0=ps[0:64, :],
                                        scalar1=bqk_sb[2][0:64, :])
            nc.vector.tensor_scalar_add(out=kB[:, nsl], in0=ps[64:128, :],
                                        scalar1=bqk_sb[2][64:128, :])

    def v_group(ii, n):
        # token tile t = 4n+ii: v^T(t) = x_tile @ wv -> [128 tokens, 192]
        t = 4 * n + ii
        ps = st_tile([128, 192], f"pv{rep}_{t}")
        for k in range(6):
            nc.tensor.matmul(ps[:], lhsT=xT_t[n][:, k, 128 * ii:128 * (ii + 1)],
                             rhs=wv_sb[:, k, :], start=(k == 0), stop=(k == 5))
        nc.vector.tensor_copy(
            out=v_all[:, t, :, 0:64],
            in_=ps[:].rearrange("p (h d) -> p h d", h=3))

    def proj_chunk(n):
        for m in range(3):
            qk_group(m, n)
        for ii in range(4):
            v_group(ii, n)

    # ---- attention ----
    def KQ(h):
        if h == 0:
            return kA[0:64], qA[0:64]
        if h == 1:
            return kA[64:128], qA[64:128]
        return kB[:], qB[:]

    pts = {}    # (h, q, p) -> pt tile
    pvps = {}   # q -> [per-head psum tile [128, 4, 128] f32 (qc, dims|l)]
    pvst = {}   # (q, h) -> started flag for psum zero-region

    def diag_mask(pt, c0):
        # zero pt[:, c0:c0+128] where key row p > query col offset c
        nc.gpsimd.affine_select(
            out=pt[:, c0:c0 + 128], in_=pt[:, c0:c0 + 128],
            compare_op=GE, fill=0.0,
            base=0, channel_multiplier=-1, pattern=[[1, 128]])

    def qk_unit(h, qq, p):
        """Score tile + exp for unit p of quarter qq, head h."""
        Kh, Qh = KQ(h)
        base = 512 * qq
        nfull = 2 * qq

        def kblk(i):
            return Kh[:, 128 * i:128 * (i + 1)]

        def qrng(a, b):
            return Qh[:, base + a:base + b]

        if p < nfull:
            # full pair: key blocks 2p, 2p+1, all 512 queries
            st = st_tile([128, 1024], f"st{rep}_{h}{qq}{p}")
            if "2" in kdup and h < 2:
                # probe: same matmul on the OPPOSITE row half, spare bank
                Kf, Qf = KQ(1 - h)
                dps = psp.tile([128, 512], F32, name=f"dp{rep}_{h}{qq}{p}",
                               tag="bcp")
                nc.tensor.matmul(dps[:], lhsT=Kf[:, 128 * 2 * p:128 * (2 * p + 1)],
                                 rhs=Qf[:, 512 * qq:512 * qq + 512],
                                 start=True, stop=True, skip_group_check=True)
            for _ in range(2 if "q" in kdup else 1):
                nc.tensor.matmul(st[:, 0:512], lhsT=kblk(2 * p), rhs=qrng(0, 512),
                                 start=True, stop=True, skip_group_check=True)
                nc.tensor.matmul(st[:, 512:1024], lhsT=kblk(2 * p + 1),
                                 rhs=qrng(0, 512), start=True, stop=True,
                                 skip_group_check=True)
            pt = sbp.tile([128, 1024], BF16, name=f"pF{rep}_{h}{qq}{p}",
                          tag=f"pF{h}_{qq % 2}_{p}")
            nc.scalar.activation(out=pt[:], in_=st[:], func=Exp,
                                 scale=float(SCALE))
            if "e" in kdup:
                dmy = sbp.tile([128, 1024], BF16, name=f"dm{rep}_{h}{qq}{p}",
                               tag=f"dm{p % 2}")
                nc.scalar.activation(out=dmy[:], in_=st[:], func=Exp,
                                     scale=float(SCALE))
            pts[(h, qq, p)] = pt
        elif p == nfull:
            # D1: bank A = [A-main 384 | A-diag 128], bank B = [B 384 | C-solo 128]
            iA = 4 * qq
            st = st_tile([128, 1024], f"sd{rep}_{h}{qq}")
            nc.tensor.matmul(st[:, 0:384], lhsT=kblk(iA), rhs=qrng(128, 512),
                             start=True, stop=False)
            nc.tensor.matmul(st[:, 384:512], lhsT=kblk(iA), rhs=qrng(0, 128),
                             start=False, stop=True)
            nc.tensor.matmul(st[:, 512:896], lhsT=kblk(iA + 1), rhs=qrng(128, 512),
                             start=True, stop=False)
            nc.tensor.matmul(st[:, 896:1024], lhsT=kblk(iA + 2), rhs=qrng(256, 384),
                             start=False, stop=True)
            pt = sbp.tile([128, 1024], BF16, name=f"pD1{rep}_{h}{qq}",
                          tag=f"pD1{h}_{qq % 2}")
            nc.scalar.activation(out=pt[:], in_=st[:], func=Exp,
                                 scale=float(SCALE))
            diag_mask(pt, 384)        # A's diagonal (queries 0:128)
            diag_mask(pt, 512)        # B's diagonal (queries 128:256)
            diag_mask(pt, 896)        # C's diagonal (queries 256:384)
            pts[(h, qq, p)] = pt
        else:
            # D2: [C-main 128 | D 128]
            iA = 4 * qq
            st = st_tile([128, 256], f"s2{rep}_{h}{qq}")
            nc.tensor.matmul(st[:, 0:128], lhsT=kblk(iA + 2), rhs=qrng(384, 512),
                             start=True, stop=False)
            nc.tensor.matmul(st[:, 128:256], lhsT=kblk(iA + 3), rhs=qrng(384, 512),
                             start=False, stop=True)
            pt = sbp.tile([128, 256], BF16, name=f"pD2{rep}_{h}{qq}",
                          tag=f"pD2{h}_{qq % 2}")
            nc.scalar.activation(out=pt[:], in_=st[:], func=Exp,
                                 scale=float(SCALE))
            diag_mask(pt, 128)        # D's diagonal (queries 384:512)
            pts[(h, qq, p)] = pt

    def pv_unit(h, pvq, p):
        """Flipped PV: pt chunks are STATIONARY ([128 keys, 128 queries]),
        v_all [128, 65] streams -> psum [128 queries, 4qc, 65] per head.
        Streams 65 cols/block-chunk instead of up-to-512: PV runs at the
        output-element bound (8,840 cycles/head vs 17,408)."""
        pvp = pvps[pvq][h]
        nfull = 2 * pvq
        pt = pts.pop((h, pvq, p))

        def mm(qc, c0, i, stop=False):
            st = not pvst.get((pvq, h), False)
            pvst[(pvq, h)] = True
            nc.tensor.matmul(pvp[:, qc, 0:65], lhsT=pt[:, c0:c0 + 128],
                             rhs=v_all[:, i, h, 0:65], start=st, stop=stop)

        if p < nfull:
            for j in range(2):
                for qc in range(4):
                    mm(qc, 512 * j + 128 * qc, 2 * p + j)
        elif p == nfull:
            iA = 4 * pvq
            for qc in (1, 2, 3):
                mm(qc, 128 * (qc - 1), iA)              # A-main
            mm(0, 384, iA)                              # A-diag
            for qc in (1, 2, 3):
                mm(qc, 512 + 128 * (qc - 1), iA + 1)    # B
            mm(2, 896, iA + 2)                          # C-solo
        else:
            iA = 4 * pvq
            mm(3, 0, iA + 2)                            # C-main
            mm(3, 128, iA + 3, stop=True)               # D

    def normalize(q):
        """l sits on the 65th column per query-partition: reciprocal +
        per-partition scale on DVE, then PE-transpose back to dim-major
        ot01/ot2 for the output projection."""
        pvp = pvps.pop(q)
        otns, rcs = [], []
        for qc in range(4):
            t = 4 * q + qc
            rc = sbp.tile([128, 3], F32, name=f"rc{rep}_{t}", tag=f"rc{qc}")
            otn = sbp.tile([128, 3, 64], BF16, name=f"on{rep}_{t}",
                           tag=f"on{qc}")
            for h in range(3):
                with nc.allow_low_precision(reason="f32 recip"):
                    nc.vector.reciprocal(out=rc[:, h:h + 1],
                                         in_=pvp[h][:, qc, 64:65])
            rcs.append(rc)
            otns.append(otn)
        for qc in range(4):
            for h in range(3):
                nc.vector.tensor_scalar_mul(out=otns[qc][:, h, :],
                                            in0=pvp[h][:, qc, 0:64],
                                            scalar1=rcs[qc][:, h:h + 1])
        for qc in range(4):
            t = 4 * q + qc
            tsl = slice(128 * t, 128 * (t + 1))
            dsts = [ot01[0:64, tsl], ot01[64:128, tsl], ot2[0:64, tsl]]
            for h in range(3):
                tp = psp.tile([64, 128], BF16, name=f"tp{rep}_{t}{h}",
                              tag=f"pv{h}")
                nc.tensor.transpose(tp[:], otns[qc][:, h, :], ident_b[:])
                nc.vector.tensor_copy(out=dsts[h], in_=tp[:])

    def phase3_tile(i):
        # out[128i:128i+128, :] = [ot01; ot2][:, tile i].T @ wf
        # Two sequential passes through the bcp bank (no pv-tag use, so the
        # tiles can interleave INTO the exp-paced streams where PE has slack
        # and DVE is idle).
        csl = slice(128 * i, 128 * (i + 1))
        ob = sbp.tile([128, EMBED_DIM], F16, name=f"ob{rep}_{i}",
                      tag=f"ob{i % 3}")
        for (c0, c1) in ((0, 512), (512, 768)):
            fp = psp.tile([128, c1 - c0], F32, name=f"fp{rep}_{i}_{c0}",
                          tag="bcp")
            nc.tensor.matmul(fp[:], lhsT=ot01[:, csl],
                             rhs=wf01_sb[:, c0:c1], start=True, stop=False)
            nc.tensor.matmul(fp[:], lhsT=ot2[:, csl],
                             rhs=wf2_sb[:, c0:c1], start=False, stop=True)
            nc.vector.tensor_copy(out=ob[:, c0:c1], in_=fp[:])
        dmae[i % 2].dma_start(out=out_d[128 * i:128 * (i + 1), :], in_=ob[:])

    # ---- emission closures ----
    def prologue():
        emit_xt_dma()
        proj_chunk(0)
        for p in range(2):          # quarter 0 bootstrap: D1, D2
            for h in range(3):
                qk_unit(h, 0, p)

    def main(next_prologue=None):
        # normalize(q)/phase3(q) are deferred into step q+1, AFTER its
        # projection chunk: the proj matmuls sit ahead of the recip/bcast
        # chain in the in-order PE queue, hiding the chain's DVE latency.
        pending = None
        for q in range(4):
            if q == 3 and next_prologue is not None:
                next_prologue()
            if q < 3:
                proj_chunk(q + 1)
            if pending is not None:
                normalize(pending)
                if q == 3:
                    # PV-only stream has no exp pacing: keep phase3 here,
                    # shielded by next_prologue's projection burst
                    for i in range(4 * pending, 4 * pending + 4):
                        phase3_tile(i)
            qq = q + 1 if q < 3 else None
            pvps[q] = [psp.tile([128, 4, 128], F32, name=f"pv{rep}_{q}{h}",
                                tag=f"pv{h}") for h in range(3)]
            nqk = 2 * (q + 1) + 2 if qq is not None else 0
            npv = 2 * q + 2
            for p in range(max(nqk, npv)):
                for h in range(3):
                    if p < nqk:
                        qk_unit(h, qq, p)
                    if p < npv:
                        pv_unit(h, q, p)
                if pending is not None and q < 3 and p < 4:
                    phase3_tile(4 * pending + p)
            pending = q
        normalize(3)
        for i in range(12, 16):
            phase3_tile(i)

    return prologue, main


def _prep_inputs(x, w_qkv, b_qkv, w_final):
    """Build the 8 per-core input maps from the full inputs."""
    import ml_dtypes
    fp8 = ml_dtypes.float8_e4m3
    bf16 = ml_dtypes.bfloat16

    x = np.asarray(x, dtype=np.float32)
    w_qkv = np.asarray(w_qkv, dtype=np.float32)
    b_qkv = np.asarray(b_qkv, dtype=np.float32)
    w_final = np.asarray(w_final, dtype=np.float32)
    E = EMBED_DIM

    in_maps = []
    for c in range(N_CORES):
        b = c // 4
        g = c % 4
        heads = [3 * g, 3 * g + 1, 3 * g + 2]
        hr = [np.arange(64 * h, 64 * h + 64) for h in heads]
        # [q0 q1 | k0 k1 | q2 | k2]
        rows_qk = np.concatenate([hr[0], hr[1], E + hr[0], E + hr[1], hr[2], E + hr[2]])
        rows_v = np.concatenate(hr) + 2 * E
        xT = np.ascontiguousarray(x[b].T).astype(bf16)               # [768, 2048]
        wqk = np.ascontiguousarray(w_qkv[rows_qk].T).astype(bf16)    # [768, 384]
        wv = np.ascontiguousarray(w_qkv[rows_v].T).astype(bf16)      # [768, 192]
        bqk = np.ascontiguousarray(b_qkv[rows_qk][:, None])
        wf = np.ascontiguousarray(w_final[:, np.concatenate(hr)].T).astype(bf16)
        in_maps.append({"xT": xT, "wqk": wqk, "wv": wv, "bqk": bqk, "wf": wf})
    return in_maps


def kernel(x, w_qkv, b_qkv, w_final, _trace=False):
    from concourse.bass_utils import run_bass_kernel_spmd

    if "nc" not in _state:
        _state["nc"] = _build()
    nc = _state["nc"]

    in_maps = _prep_inputs(x, w_qkv, b_qkv, w_final)
    res = run_bass_kernel_spmd(nc, in_maps, list(range(N_CORES)), trace=_trace)
    _state["last_result"] = res

    w_final = np.asarray(w_final, dtype=np.float64)
    b_qkv = np.asarray(b_qkv, dtype=np.float64)
    # v bias folds into a constant row: softmax weights sum to 1
    const_row = w_final @ b_qkv[2 * EMBED_DIM:]

    out = np.empty((B, T, EMBED_DIM), dtype=np.float32)
    for b in range(B):
        acc = np.zeros((T, EMBED_DIM), dtype=np.float64)
        for g in range(4):
            acc += res.results[4 * b + g]["out_p"].astype(np.float64)
        out[b] = (acc + const_row).astype(np.float32)
    return out


# revision 31
# speedup vs baseline: 1.2229x; 1.0256x over previous
"""Multi-head causal attention (B=2, T=2048, E=768, H=12, D=64) on 8 trn2 cores.

Sharding: core c handles batch b=c//4 and heads [3g, 3g+1, 3g+2] (g=c%4).
Each core computes its 3 heads' attention plus their partial contribution to
the final projection; the host sums the 4 partials per batch.

Differences vs the original transpose-based kernel:
- V is projected directly in [tokens, dims] orientation (lhsT = x token
  tile), eliminating all 48 PE transposes (-12k PE cycles/body); its bias is
  folded into a host-side constant row (softmax weights sum to 1).
- Unified per-head attention over key-block PAIRS, bank-aligned: full pair
  tiles are [block 2p | block 2p+1] (1024 cols, ONE exp); the 4 diagonal
  blocks A..D of each quarter pack into two tiles:
    st1 (2 banks): [A-main 384 | A-diag 128 | B 384 | C-solo 128]
    st2 (1 bank):  [C-main 128 | D 128]
  Every matmul output stays inside a 2KB psum bank (hard hw rule) and each
  tile needs one exp (60 exps/body).  Diagonal 128-blocks are masked AFTER
  exp by gpsimd affine_select on the SBUF pt tiles.
- FLIPPED PV: the exp'd score chunks ([128 keys, 128 queries]) are the
  STATIONARY operand and v_all [128, 65] streams, accumulating O^T per
  query-chunk into psum [128 queries, 4, 65|l].  PV streams 65 cols per
  (block, chunk) = the output-element bound: 8,840 cycles/head vs 17,408
  for the classic orientation (which is moving-column-bound on pt).
  Bonus: l lands on the PARTITION axis, so normalize is a plain DVE
  reciprocal + per-partition tensor_scalar (the whole PE-broadcast /
  copy machinery is gone); PE transposes (16 tiles x 3 heads) restore
  dim-major ot01/ot2 for the output projection.
- normalize(q) is deferred into step q+1 AFTER its projection chunk, so
  the recip/mul (DVE) + transpose (PE) chain hides behind ~6k cycles of
  independent projection matmuls in the in-order PE queue.  phase3(q)'s
  tiles then interleave INTO step q+1's exp-paced stream (one tile per
  stream step): the stream has PE slack and an idle DVE, so the output
  projection rides along for ~free (bcp-bank sequential, no pv-tag use).
  The q=3 stream is PV-only (no exp pacing), so phase3(2) stays pre-stream
  shielded by next_prologue's projection burst.
- Output partials are DMA'd as fp16 (halves output HBM traffic; summed in
  f64 on the host).

Everything numeric is bf16 into f32 psum.  fp8(e4m3) + DoubleRow was built
and measured: it passes BIR/hardware fine (see git-less probe history) but
any fp8 tensor in the q/k path, v, or even just the output projection blows
the rel-2e-2 max-norm budget (exp amplifies score jitter; peaked softmax rows
expose raw v quantization; max over 3.1M outputs sits ~5 sigma up), so it is
not used.

Measured: ~114us/body (vs 171us for the classic-orientation kernel).
PE-cycle-bound; ~165k PE cycles/body (proj 55k, QK 52k, PV 26.5k,
transposes 6k, out-proj 25k).  KDUP=q probes measured ~1.13GHz marginal in
the OLD long-stream regime, yet this kernel beats the 1.2GHz pure-stream
bound -- the clock gate rewards the flipped PV's short clean streams, so
treat clock models as advisory and measure.  ACT (exp, 55us) and DVE have
slack.  Row-tile co-execution does NOT exist (probe_rows.py: packed/serial
= 0.73, just overhead hiding), and fp8/DoubleRow is numerically dead at
this tolerance everywhere.

PSUM banks (8): stA0/stA1 [128,1024] f32 (2 each) + otl0/1/2 [128,512] + bcp.
`repeat` unrolls the body N times in one NEFF; test.py measures per-body HW
time as the slope of wall time vs N.  KDUP=q/e/2 add duplicate work for
differential load probing (default off).
"""
import numpy as np

EMBED_DIM = 768
B = 2
T = 2048
N_CORES = 8
NT = T // 128           # 16 key/query tiles
SCALE = 1.0 / np.sqrt(64.0)

_state = {}


def _build(repeat=1):
    import concourse.tile as tile
    from concourse import bacc, mybir
    from concourse.masks import make_identity

    F32 = mybir.dt.float32
    F32R = mybir.dt.float32r
    BF16 = mybir.dt.bfloat16
    FP8 = mybir.dt.float8e4
    F16 = mybir.dt.float16

    nc = bacc.Bacc("TRN2", target_bir_lowering=False, debug=False)

    xT_d = nc.dram_tensor("xT", [EMBED_DIM, T], BF16, kind="ExternalInput").ap()
    # columns ordered [q0 q1 | k0 k1 | q2 | k2]
    wqk_d = nc.dram_tensor("wqk", [EMBED_DIM, 384], BF16, kind="ExternalInput").ap()
    wv_d = nc.dram_tensor("wv", [EMBED_DIM, 192], BF16, kind="ExternalInput").ap()
    bqk_d = nc.dram_tensor("bqk", [384, 1], F32, kind="ExternalInput").ap()
    # w_final^T rows for this core's 192 dims: [0:128) and [128:192)
    wf_d = nc.dram_tensor("wf", [192, EMBED_DIM], BF16, kind="ExternalInput").ap()
    out_d = nc.dram_tensor("out_p", [T, EMBED_DIM], F16, kind="ExternalOutput").ap()

    with tile.TileContext(nc) as tc:
        with tc.tile_pool(name="const", bufs=1) as const, \
             tc.tile_pool(name="persist", bufs=1) as persist, \
             tc.tile_pool(name="sbod", bufs=1) as sbp, \
             tc.tile_pool(name="psod", bufs=1, space="PSUM") as psp:
            # ---- constants ----
            wqk_sb = const.tile([128, 6, 384], BF16)
            wv_sb = const.tile([128, 6, 192], BF16)
            nc.sync.dma_start(out=wqk_sb[:], in_=wqk_d.rearrange("(k p) c -> p k c", p=128))
            nc.gpsimd.dma_start(out=wv_sb[:], in_=wv_d.rearrange("(k p) c -> p k c", p=128))
            bqk_sb = [const.tile([128, 1], F32, name=f"bqk{m}", tag=f"bqk{m}")
                      for m in range(3)]
            for m in range(3):
                nc.sync.dma_start(out=bqk_sb[m][:], in_=bqk_d[128 * m:128 * (m + 1), :])
            wf01_sb = const.tile([128, EMBED_DIM], BF16)
            wf2_sb = const.tile([64, EMBED_DIM], BF16)
            nc.gpsimd.dma_start(out=wf01_sb[:], in_=wf_d[0:128, :])
            nc.gpsimd.dma_start(out=wf2_sb[:], in_=wf_d[128:192, :])
            ident_f = const.tile([128, 128], F32)
            make_identity(nc, ident_f)
            ident_b = const.tile([128, 128], BF16)
            nc.vector.tensor_copy(out=ident_b[:], in_=ident_f[:])

            # ---- persistent activations ----
            qA = persist.tile([128, T], BF16)   # q0 @0:64, q1 @64:128
            kA = persist.tile([128, T], BF16)   # k0 @0:64, k1 @64:128
            qB = persist.tile([64, T], BF16)    # q2
            kB = persist.tile([64, T], BF16)    # k2
            # [v | 1] per (key tile, head).  Double-buffered by body parity
            # (the next body's projection writes them while this body's
            # final-quarter PVs still read).
            v_alls = [persist.tile([128, NT, 3, 66], BF16, name=f"v_all{p}",
                                   tag=f"v_all{p}") for p in range(2)]
            for p in range(2):
                nc.vector.memset(v_alls[p][:], 1.0)   # col 64 stays 1.0
            ot01 = persist.tile([128, T], BF16)  # heads 0 (@0:64) & 1 (@64:128)
            ot2 = persist.tile([64, T], BF16)    # head 2

            env = locals()
            bodies = [_make_body(nc, tc, rep, env) for rep in range(repeat)]
            bodies[0][0]()
            for rep in range(repeat):
                nxt = bodies[rep + 1][0] if rep + 1 < repeat else None
                bodies[rep][1](nxt)

    nc.compile()
    return nc


def _make_body(nc, tc, rep, env):
    """Build one body's emission closures; returns (prologue, main)."""
    import os
    from concourse import mybir
    kdup = os.environ.get("KDUP", "")

    F32 = mybir.dt.float32
    F32R = mybir.dt.float32r
    BF16 = mybir.dt.bfloat16
    F16 = mybir.dt.float16
    Exp = mybir.ActivationFunctionType.Exp
    MULT = mybir.AluOpType.mult
    GE = mybir.AluOpType.is_ge
    DR = mybir.MatmulPerfMode.DoubleRow

    xT_d, out_d = env["xT_d"], env["out_d"]
    wqk_sb, wv_sb = env["wqk_sb"], env["wv_sb"]
    bqk_sb = env["bqk_sb"]
    wf01_sb, wf2_sb = env["wf01_sb"], env["wf2_sb"]
    ident_b = env["ident_b"]
    qA, kA, qB, kB = env["qA"], env["kA"], env["qB"], env["kB"]
    v_all = env["v_alls"][rep % 2]
    ot01, ot2 = env["ot01"], env["ot2"]
    dmae = [nc.sync, nc.gpsimd]

    sbp, psp = env["sbp"], env["psp"]

    # ---- input DMA: xT as 4 chunks of [128, 6, 512] ----
    xT_t = [sbp.tile([128, 6, 512], BF16, name=f"xT{rep}_{n}", tag=f"xT{n}")
            for n in range(4)]

    def emit_xt_dma():
        for n in range(4):
            for k in range(6):
                nc.sync.dma_start(
                    out=xT_t[n][:, k, :],
                    in_=xT_d[128 * k:128 * (k + 1), 512 * n:512 * (n + 1)])

    gidx = [0]

    def st_tile(shape, name):
        t = psp.tile(shape, F32, name=name, tag=f"stA{gidx[0] % 2}")
        gidx[0] += 1
        return t

    def qk_group(m, n):
        # m: 0=[q0q1]->qA, 1=[k0k1]->kA, 2=[q2|k2]->qB+kB
        c0, c1 = 128 * m, 128 * (m + 1)
        ps = st_tile([128, 512], f"pg{rep}_{m}{n}")
        for k in range(6):
            nc.tensor.matmul(ps[:], lhsT=wqk_sb[:, k, c0:c1],
                             rhs=xT_t[n][:, k, :], start=(k == 0), stop=(k == 5))
        nsl = slice(512 * n, 512 * (n + 1))
        if m < 2:
            dst = qA if m == 0 else kA
            nc.vector.tensor_scalar_add(out=dst[:, nsl], in0=ps[:],
                                        scalar1=bqk_sb[m][:])
        else:
            nc.vector.tensor_scalar_add(out=qB[:, nsl], in0=ps[0:64, :],
                                        scalar1=bqk_sb[2][0:64, :])
            nc.vector.tensor_scalar_add(out=kB[:, nsl], in0=ps[64:128, :],
                                        scalar1=bqk_sb[2][64:128, :])

    def v_group(ii, n):
        # token tile t = 4n+ii: v^T(t) = x_tile @ wv -> [128 tokens, 192]
        t = 4 * n + ii
        ps = st_tile([128, 192], f"pv{rep}_{t}")
        for k in range(6):
            nc.tensor.matmul(ps[:], lhsT=xT_t[n][:, k, 128 * ii:128 * (ii + 1)],
                             rhs=wv_sb[:, k, :], start=(k == 0), stop=(k == 5))
        nc.vector.tensor_copy(
            out=v_all[:, t, :, 0:64],
            in_=ps[:].rearrange("p (h d) -> p h d", h=3))

    def proj_chunk(n):
        for m in range(3):
            qk_group(m, n)
        for ii in range(4):
            v_group(ii, n)

    # ---- attention ----
    def KQ(h):
        if h == 0:
            return kA[0:64], qA[0:64]
        if h == 1:
            return kA[64:128], qA[64:128]
        return kB[:], qB[:]

    pts = {}    # (h, q, p) -> pt tile
    pvps = {}   # q -> [per-head psum tile [128, 4, 128] f32 (qc, dims|l)]
    pvst = {}   # (q, h) -> started flag for psum zero-region

    def diag_mask(pt, c0):
        # zero pt[:, c0:c0+128] where key row p > query col offset c
        nc.gpsimd.affine_select(
            out=pt[:, c0:c0 + 128], in_=pt[:, c0:c0 + 128],
            compare_op=GE, fill=0.0,
            base=0, channel_multiplier=-1, pattern=[[1, 128]])

    def qk_unit(h, qq, p):
        """Score tile + exp for unit p of quarter qq, head h."""
        Kh, Qh = KQ(h)
        base = 512 * qq
        nfull = 2 * qq

        def kblk(i):
            return Kh[:, 128 * i:128 * (i + 1)]

        def qrng(a, b):
            return Qh[:, base + a:base + b]

        if p < nfull:
            # full pair: key blocks 2p, 2p+1, all 512 queries
            st = st_tile([128, 1024], f"st{rep}_{h}{qq}{p}")
            if "2" in kdup and h < 2:
                # probe: same matmul on the OPPOSITE row half, spare bank
                Kf, Qf = KQ(1 - h)
                dps = psp.tile([128, 512], F32, name=f"dp{rep}_{h}{qq}{p}",
                               tag="bcp")
                nc.tensor.matmul(dps[:], lhsT=Kf[:, 128 * 2 * p:128 * (2 * p + 1)],
                                 rhs=Qf[:, 512 * qq:512 * qq + 512],
                                 start=True, stop=True, skip_group_check=True)
            for _ in range(2 if "q" in kdup else 1):
                nc.tensor.matmul(st[:, 0:512], lhsT=kblk(2 * p), rhs=qrng(0, 512),
                                 start=True, stop=True, skip_group_check=True)
                nc.tensor.matmul(st[:, 512:1024], lhsT=kblk(2 * p + 1),
                                 rhs=qrng(0, 512), start=True, stop=True,
                                 skip_group_check=True)
            pt = sbp.tile([128, 1024], BF16, name=f"pF{rep}_{h}{qq}{p}",
                          tag=f"pF{h}_{qq % 2}_{p}")
            nc.scalar.activation(out=pt[:], in_=st[:], func=Exp,
                                 scale=float(SCALE))
            if "e" in kdup:
                dmy = sbp.tile([128, 1024], BF16, name=f"dm{rep}_{h}{qq}{p}",
                               tag=f"dm{p % 2}")
                nc.scalar.activation(out=dmy[:], in_=st[:], func=Exp,
                                     scale=float(SCALE))
            pts[(h, qq, p)] = pt
        elif p == nfull:
            # D1: bank A = [A-main 384 | A-diag 128], bank B = [B 384 | C-solo 128]
            iA = 4 * qq
            st = st_tile([128, 1024], f"sd{rep}_{h}{qq}")
            nc.tensor.matmul(st[:, 0:384], lhsT=kblk(iA), rhs=qrng(128, 512),
                             start=True, stop=False)
            nc.tensor.matmul(st[:, 384:512], lhsT=kblk(iA), rhs=qrng(0, 128),
                             start=False, stop=True)
            nc.tensor.matmul(st[:, 512:896], lhsT=kblk(iA + 1), rhs=qrng(128, 512),
                             start=True, stop=False)
            nc.tensor.matmul(st[:, 896:1024], lhsT=kblk(iA + 2), rhs=qrng(256, 384),
                             start=False, stop=True)
            pt = sbp.tile([128, 1024], BF16, name=f"pD1{rep}_{h}{qq}",
                          tag=f"pD1{h}_{qq % 2}")
            nc.scalar.activation(out=pt[:], in_=st[:], func=Exp,
                                 scale=float(SCALE))
            diag_mask(pt, 384)        # A's diagonal (queries 0:128)
            diag_mask(pt, 512)        # B's diagonal (queries 128:256)
            diag_mask(pt, 896)        # C's diagonal (queries 256:384)
            pts[(h, qq, p)] = pt
        else:
            # D2: [C-main 128 | D 128]
            iA = 4 * qq
            st = st_tile([128, 256], f"s2{rep}_{h}{qq}")
            nc.tensor.matmul(st[:, 0:128], lhsT=kblk(iA + 2), rhs=qrng(384, 512),
                             start=True, stop=False)
            nc.tensor.matmul(st[:, 128:256], lhsT=kblk(iA + 3), rhs=qrng(384, 512),
                             start=False, stop=True)
            pt = sbp.tile([128, 256], BF16, name=f"pD2{rep}_{h}{qq}",
                          tag=f"pD2{h}_{qq % 2}")
            nc.scalar.activation(out=pt[:], in_=st[:], func=Exp,
                                 scale=float(SCALE))
            diag_mask(pt, 128)        # D's diagonal (queries 384:512)
            pts[(h, qq, p)] = pt

    def pv_unit(h, pvq, p):
        """Flipped PV: pt chunks are STATIONARY ([128 keys, 128 queries]),
        v_all [128, 65] streams -> psum [128 queries, 4qc, 65] per head.
        Streams 65 cols/block-chunk instead of up-to-512: PV runs at the
        output-element bound (8,840 cycles/head vs 17,408)."""
        pvp = pvps[pvq][h]
        nfull = 2 * pvq
        pt = pts.pop((h, pvq, p))

        def mm(qc, c0, i, stop=False):
            st = not pvst.get((pvq, h), False)
            pvst[(pvq, h)] = True
            nc.tensor.matmul(pvp[:, qc, 0:65], lhsT=pt[:, c0:c0 + 128],
                             rhs=v_all[:, i, h, 0:65], start=st, stop=stop)

        if p < nfull:
            for j in range(2):
                for qc in range(4):
                    mm(qc, 512 * j + 128 * qc, 2 * p + j)
        elif p == nfull:
            iA = 4 * pvq
            for qc in (1, 2, 3):
                mm(qc, 128 * (qc - 1), iA)              # A-main
            mm(0, 384, iA)                              # A-diag
            for qc in (1, 2, 3):
                mm(qc, 512 + 128 * (qc - 1), iA + 1)    # B
            mm(2, 896, iA + 2)                          # C-solo
        else:
            iA = 4 * pvq
            mm(3, 0, iA + 2)                            # C-main
            mm(3, 128, iA + 3, stop=True)               # D

    def normalize(q):
        """l sits on the 65th column per query-partition: reciprocal +
        per-partition scale on DVE, then PE-transpose back to dim-major
        ot01/ot2 for the output projection."""
        pvp = pvps.pop(q)
        otns, rcs = [], []
        for qc in range(4):
            t = 4 * q + qc
            rc = sbp.tile([128, 3], F32, name=f"rc{rep}_{t}", tag=f"rc{qc}")
            otn = sbp.tile([128, 3, 64], BF16, name=f"on{rep}_{t}",
                           tag=f"on{qc}")
            for h in range(3):
                with nc.allow_low_precision(reason="f32 recip"):
                    nc.vector.reciprocal(out=rc[:, h:h + 1],
                                         in_=pvp[h][:, qc, 64:65])
            rcs.append(rc)
            otns.append(otn)
        for qc in range(4):
            for h in range(3):
                nc.vector.tensor_scalar_mul(out=otns[qc][:, h, :],
                                            in0=pvp[h][:, qc, 0:64],
                                            scalar1=rcs[qc][:, h:h + 1])
        for qc in range(4):
            t = 4 * q + qc
            tsl = slice(128 * t, 128 * (t + 1))
            dsts = [ot01[0:64, tsl], ot01[64:128, tsl], ot2[0:64, tsl]]
            for h in range(3):
                tp = psp.tile([64, 128], BF16, name=f"tp{rep}_{t}{h}",
                              tag=f"pv{h}")
                nc.tensor.transpose(tp[:], otns[qc][:, h, :], ident_b[:])
                nc.vector.tensor_copy(out=dsts[h], in_=tp[:])

    def phase3_tile(i):
        # out[128i:128i+128, :] = [ot01; ot2][:, tile i].T @ wf
        # Two sequential passes through the bcp bank (no pv-tag use, so the
        # tiles can interleave INTO the exp-paced streams where PE has slack
        # and DVE is idle).
        csl = slice(128 * i, 128 * (i + 1))
        ob = sbp.tile([128, EMBED_DIM], F16, name=f"ob{rep}_{i}",
                      tag=f"ob{i % 3}")
        for (c0, c1) in ((0, 512), (512, 768)):
            fp = psp.tile([128, c1 - c0], F32, name=f"fp{rep}_{i}_{c0}",
                          tag="bcp")
            nc.tensor.matmul(fp[:], lhsT=ot01[:, csl],
                             rhs=wf01_sb[:, c0:c1], start=True, stop=False)
            nc.tensor.matmul(fp[:], lhsT=ot2[:, csl],
                             rhs=wf2_sb[:, c0:c1], start=False, stop=True)
            nc.vector.tensor_copy(out=ob[:, c0:c1], in_=fp[:])
        dmae[i % 2].dma_start(out=out_d[128 * i:128 * (i + 1), :], in_=ob[:])

    # ---- emission closures ----
    def prologue():
        emit_xt_dma()
        for m in range(3):
            qk_group(m, 0)
        for p in range(2):          # quarter 0 bootstrap: D1, D2
            for h in range(3):
                qk_unit(h, 0, p)
            v_group(2 * p, 0)
            v_group(2 * p + 1, 0)

    def main(next_prologue=None):
        # normalize(q)/phase3(q) are deferred into step q+1, AFTER its
        # projection chunk: the proj matmuls sit ahead of the recip/bcast
        # chain in the in-order PE queue, hiding the chain's DVE latency.
        pending = None
        for q in range(4):
            if q == 3 and next_prologue is not None:
                next_prologue()
            if q < 3:
                for m in range(3):
                    qk_group(m, q + 1)      # feeds THIS stream's QK
            if pending is not None:
                normalize(pending)
                if q == 3:
                    # PV-only stream has no exp pacing: keep phase3 here,
                    # shielded by next_prologue's projection burst
                    for i in range(4 * pending, 4 * pending + 4):
                        phase3_tile(i)
            qq = q + 1 if q < 3 else None
            pvps[q] = [psp.tile([128, 4, 128], F32, name=f"pv{rep}_{q}{h}",
                                tag=f"pv{h}") for h in range(3)]
            nqk = 2 * (q + 1) + 2 if qq is not None else 0
            npv = 2 * q + 2
            for p in range(max(nqk, npv)):
                for h in range(3):
                    if p < nqk:
                        qk_unit(h, qq, p)
                    if p < npv:
                        pv_unit(h, q, p)
                if pending is not None and q < 3 and p < 4:
                    phase3_tile(4 * pending + p)
                if q < 3 and p < 4:
                    # chunk q+1's V rides in-stream: its psum drains via an
                    # idle-DVE copy, and PV(q) only reads older v tiles
                    v_group(p, q + 1)
            pending = q
        normalize(3)
        for i in range(12, 16):
            phase3_tile(i)

    return prologue, main


def _prep_inputs(x, w_qkv, b_qkv, w_final):
    """Build the 8 per-core input maps from the full inputs."""
    import ml_dtypes
    fp8 = ml_dtypes.float8_e4m3
    bf16 = ml_dtypes.bfloat16

    x = np.asarray(x, dtype=np.float32)
    w_qkv = np.asarray(w_qkv, dtype=np.float32)
    b_qkv = np.asarray(b_qkv, dtype=np.float32)
    w_final = np.asarray(w_final, dtype=np.float32)
    E = EMBED_DIM

    in_maps = []
    for c in range(N_CORES):
        b = c // 4
        g = c % 4
        heads = [3 * g, 3 * g + 1, 3 * g + 2]
        hr = [np.arange(64 * h, 64 * h + 64) for h in heads]
        # [q0 q1 | k0 k1 | q2 | k2]
        rows_qk = np.concatenate([hr[0], hr[1], E + hr[0], E + hr[1], hr[2], E + hr[2]])
        rows_v = np.concatenate(hr) + 2 * E
        xT = np.ascontiguousarray(x[b].T).astype(bf16)               # [768, 2048]
        wqk = np.ascontiguousarray(w_qkv[rows_qk].T).astype(bf16)    # [768, 384]
        wv = np.ascontiguousarray(w_qkv[rows_v].T).astype(bf16)      # [768, 192]
        bqk = np.ascontiguousarray(b_qkv[rows_qk][:, None])
        wf = np.ascontiguousarray(w_final[:, np.concatenate(hr)].T).astype(bf16)
        in_maps.append({"xT": xT, "wqk": wqk, "wv": wv, "bqk": bqk, "wf": wf})
    return in_maps


def kernel(x, w_qkv, b_qkv, w_final, _trace=False):
    from concourse.bass_utils import run_bass_kernel_spmd

    if "nc" not in _state:
        _state["nc"] = _build()
    nc = _state["nc"]

    in_maps = _prep_inputs(x, w_qkv, b_qkv, w_final)
    res = run_bass_kernel_spmd(nc, in_maps, list(range(N_CORES)), trace=_trace)
    _state["last_result"] = res

    w_final = np.asarray(w_final, dtype=np.float64)
    b_qkv = np.asarray(b_qkv, dtype=np.float64)
    # v bias folds into a constant row: softmax weights sum to 1
    const_row = w_final @ b_qkv[2 * EMBED_DIM:]

    out = np.empty((B, T, EMBED_DIM), dtype=np.float32)
    for b in range(B):
        acc = np.zeros((T, EMBED_DIM), dtype=np.float64)
        for g in range(4):
            acc += res.results[4 * b + g]["out_p"].astype(np.float64)
        out[b] = (acc + const_row).astype(np.float32)
    return out


# revision 32
# speedup vs baseline: 1.2351x; 1.0100x over previous
"""Multi-head causal attention (B=2, T=2048, E=768, H=12, D=64) on 8 trn2 cores.

Sharding: core c handles batch b=c//4 and heads [3g, 3g+1, 3g+2] (g=c%4).
Each core computes its 3 heads' attention plus their partial contribution to
the final projection; the host sums the 4 partials per batch.

Differences vs the original transpose-based kernel:
- V is projected directly in [tokens, dims] orientation (lhsT = x token
  tile), eliminating all 48 PE transposes (-12k PE cycles/body); its bias is
  folded into a host-side constant row (softmax weights sum to 1).
- Unified per-head attention over key-block PAIRS, bank-aligned: full pair
  tiles are [block 2p | block 2p+1] (1024 cols, ONE exp); the 4 diagonal
  blocks A..D of each quarter pack into two tiles:
    st1 (2 banks): [A-main 384 | A-diag 128 | B 384 | C-solo 128]
    st2 (1 bank):  [C-main 128 | D 128]
  Every matmul output stays inside a 2KB psum bank (hard hw rule) and each
  tile needs one exp (60 exps/body).  Diagonal 128-blocks are masked AFTER
  exp by gpsimd affine_select on the SBUF pt tiles.
- FLIPPED PV: the exp'd score chunks ([128 keys, 128 queries]) are the
  STATIONARY operand and v_all [128, 65] streams, accumulating O^T per
  query-chunk into psum [128 queries, 4, 65|l].  PV streams 65 cols per
  (block, chunk) = the output-element bound: 8,840 cycles/head vs 17,408
  for the classic orientation (which is moving-column-bound on pt).
  Bonus: l lands on the PARTITION axis, so normalize is a plain DVE
  reciprocal + per-partition tensor_scalar (the whole PE-broadcast /
  copy machinery is gone); PE transposes (16 tiles x 3 heads) restore
  dim-major ot01/ot2 for the output projection.
- normalize(q) is deferred into step q+1 AFTER its projection chunk, so
  the recip/mul (DVE) + transpose (PE) chain hides behind ~6k cycles of
  independent projection matmuls in the in-order PE queue.  phase3(q)'s
  tiles then interleave INTO step q+1's exp-paced stream (one tile per
  stream step): the stream has PE slack and an idle DVE, so the output
  projection rides along for ~free (bcp-bank sequential, no pv-tag use).
  The q=3 stream is PV-only (no exp pacing), so phase3(2) stays pre-stream
  shielded by next_prologue's projection burst.
- Output partials are DMA'd as fp16 (halves output HBM traffic; summed in
  f64 on the host).

Everything numeric is bf16 into f32 psum.  fp8(e4m3) + DoubleRow was built
and measured: it passes BIR/hardware fine (see git-less probe history) but
any fp8 tensor in the q/k path, v, or even just the output projection blows
the rel-2e-2 max-norm budget (exp amplifies score jitter; peaked softmax rows
expose raw v quantization; max over 3.1M outputs sits ~5 sigma up), so it is
not used.

Measured: ~114us/body (vs 171us for the classic-orientation kernel).
PE-cycle-bound; ~165k PE cycles/body (proj 55k, QK 52k, PV 26.5k,
transposes 6k, out-proj 25k).  KDUP=q probes measured ~1.13GHz marginal in
the OLD long-stream regime, yet this kernel beats the 1.2GHz pure-stream
bound -- the clock gate rewards the flipped PV's short clean streams, so
treat clock models as advisory and measure.  ACT (exp, 55us) and DVE have
slack.  Row-tile co-execution does NOT exist (probe_rows.py: packed/serial
= 0.73, just overhead hiding), and fp8/DoubleRow is numerically dead at
this tolerance everywhere.

PSUM banks (8): stA0/stA1 [128,1024] f32 (2 each) + otl0/1/2 [128,512] + bcp.
`repeat` unrolls the body N times in one NEFF; test.py measures per-body HW
time as the slope of wall time vs N.  KDUP=q/e/2 add duplicate work for
differential load probing (default off).
"""
import numpy as np

EMBED_DIM = 768
B = 2
T = 2048
N_CORES = 8
NT = T // 128           # 16 key/query tiles
SCALE = 1.0 / np.sqrt(64.0)

_state = {}


def _build(repeat=1):
    import concourse.tile as tile
    from concourse import bacc, mybir
    from concourse.masks import make_identity

    F32 = mybir.dt.float32
    F32R = mybir.dt.float32r
    BF16 = mybir.dt.bfloat16
    FP8 = mybir.dt.float8e4
    F16 = mybir.dt.float16

    nc = bacc.Bacc("TRN2", target_bir_lowering=False, debug=False)

    xT_d = nc.dram_tensor("xT", [EMBED_DIM, T], BF16, kind="ExternalInput").ap()
    # columns ordered [q0 q1 | k0 k1 | q2 | k2]
    wqk_d = nc.dram_tensor("wqk", [EMBED_DIM, 384], BF16, kind="ExternalInput").ap()
    wv_d = nc.dram_tensor("wv", [EMBED_DIM, 192], BF16, kind="ExternalInput").ap()
    bqk_d = nc.dram_tensor("bqk", [384, 1], F32, kind="ExternalInput").ap()
    # w_final^T rows for this core's 192 dims: [0:128) and [128:192)
    wf_d = nc.dram_tensor("wf", [192, EMBED_DIM], BF16, kind="ExternalInput").ap()
    out_d = nc.dram_tensor("out_p", [T, EMBED_DIM], F16, kind="ExternalOutput").ap()

    with tile.TileContext(nc) as tc:
        with tc.tile_pool(name="const", bufs=1) as const, \
             tc.tile_pool(name="persist", bufs=1) as persist, \
             tc.tile_pool(name="sbod", bufs=1) as sbp, \
             tc.tile_pool(name="psod", bufs=1, space="PSUM") as psp:
            # ---- constants ----
            wqk_sb = const.tile([128, 6, 384], BF16)
            wv_sb = const.tile([128, 6, 192], BF16)
            nc.sync.dma_start(out=wqk_sb[:], in_=wqk_d.rearrange("(k p) c -> p k c", p=128))
            nc.gpsimd.dma_start(out=wv_sb[:], in_=wv_d.rearrange("(k p) c -> p k c", p=128))
            bqk_sb = [const.tile([128, 1], F32, name=f"bqk{m}", tag=f"bqk{m}")
                      for m in range(3)]
            for m in range(3):
                nc.sync.dma_start(out=bqk_sb[m][:], in_=bqk_d[128 * m:128 * (m + 1), :])
            wf01_sb = const.tile([128, EMBED_DIM], BF16)
            wf2_sb = const.tile([64, EMBED_DIM], BF16)
            nc.gpsimd.dma_start(out=wf01_sb[:], in_=wf_d[0:128, :])
            nc.gpsimd.dma_start(out=wf2_sb[:], in_=wf_d[128:192, :])
            ident_f = const.tile([128, 128], F32)
            make_identity(nc, ident_f)
            ident_b = const.tile([128, 128], BF16)
            nc.vector.tensor_copy(out=ident_b[:], in_=ident_f[:])

            # ---- persistent activations ----
            qA = persist.tile([128, T], BF16)   # q0 @0:64, q1 @64:128
            kA = persist.tile([128, T], BF16)   # k0 @0:64, k1 @64:128
            qB = persist.tile([64, T], BF16)    # q2
            kB = persist.tile([64, T], BF16)    # k2
            # [v | 1] per (key tile, head).  Double-buffered by body parity
            # (the next body's projection writes them while this body's
            # final-quarter PVs still read).
            v_alls = [persist.tile([128, NT, 3, 66], BF16, name=f"v_all{p}",
                                   tag=f"v_all{p}") for p in range(2)]
            for p in range(2):
                nc.vector.memset(v_alls[p][:], 1.0)   # col 64 stays 1.0
            ot01 = persist.tile([128, T], BF16)  # heads 0 (@0:64) & 1 (@64:128)
            ot2 = persist.tile([64, T], BF16)    # head 2

            env = locals()
            bodies = [_make_body(nc, tc, rep, env) for rep in range(repeat)]
            bodies[0][0]()
            for rep in range(repeat):
                nxt = bodies[rep + 1][0] if rep + 1 < repeat else None
                bodies[rep][1](nxt)

    nc.compile()
    return nc


def _make_body(nc, tc, rep, env):
    """Build one body's emission closures; returns (prologue, main)."""
    import os
    from concourse import mybir
    kdup = os.environ.get("KDUP", "")

    F32 = mybir.dt.float32
    F32R = mybir.dt.float32r
    BF16 = mybir.dt.bfloat16
    F16 = mybir.dt.float16
    Exp = mybir.ActivationFunctionType.Exp
    MULT = mybir.AluOpType.mult
    GE = mybir.AluOpType.is_ge
    DR = mybir.MatmulPerfMode.DoubleRow

    xT_d, out_d = env["xT_d"], env["out_d"]
    wqk_sb, wv_sb = env["wqk_sb"], env["wv_sb"]
    bqk_sb = env["bqk_sb"]
    wf01_sb, wf2_sb = env["wf01_sb"], env["wf2_sb"]
    ident_b = env["ident_b"]
    qA, kA, qB, kB = env["qA"], env["kA"], env["qB"], env["kB"]
    v_all = env["v_alls"][rep % 2]
    ot01, ot2 = env["ot01"], env["ot2"]
    dmae = [nc.sync, nc.gpsimd]

    sbp, psp = env["sbp"], env["psp"]

    # ---- input DMA: xT as 4 chunks of [128, 6, 512] ----
    xT_t = [sbp.tile([128, 6, 512], BF16, name=f"xT{rep}_{n}", tag=f"xT{n}")
            for n in range(4)]

    def emit_xt_dma():
        for n in range(4):
            for k in range(6):
                nc.sync.dma_start(
                    out=xT_t[n][:, k, :],
                    in_=xT_d[128 * k:128 * (k + 1), 512 * n:512 * (n + 1)])

    gidx = [0]

    def st_tile(shape, name):
        t = psp.tile(shape, F32, name=name, tag=f"stA{gidx[0] % 2}")
        gidx[0] += 1
        return t

    def qk_group(m, n):
        # m: 0=[q0q1]->qA, 1=[k0k1]->kA, 2=[q2|k2]->qB+kB
        c0, c1 = 128 * m, 128 * (m + 1)
        ps = st_tile([128, 512], f"pg{rep}_{m}{n}")
        for k in range(6):
            nc.tensor.matmul(ps[:], lhsT=wqk_sb[:, k, c0:c1],
                             rhs=xT_t[n][:, k, :], start=(k == 0), stop=(k == 5))
        nsl = slice(512 * n, 512 * (n + 1))
        if m < 2:
            dst = qA if m == 0 else kA
            nc.vector.tensor_scalar_add(out=dst[:, nsl], in0=ps[:],
                                        scalar1=bqk_sb[m][:])
        else:
            nc.vector.tensor_scalar_add(out=qB[:, nsl], in0=ps[0:64, :],
                                        scalar1=bqk_sb[2][0:64, :])
            nc.vector.tensor_scalar_add(out=kB[:, nsl], in0=ps[64:128, :],
                                        scalar1=bqk_sb[2][64:128, :])

    def v_group(ii, n):
        # token tile t = 4n+ii: v^T(t) = x_tile @ wv -> [128 tokens, 192]
        t = 4 * n + ii
        ps = st_tile([128, 192], f"pv{rep}_{t}")
        for k in range(6):
            nc.tensor.matmul(ps[:], lhsT=xT_t[n][:, k, 128 * ii:128 * (ii + 1)],
                             rhs=wv_sb[:, k, :], start=(k == 0), stop=(k == 5))
        nc.vector.tensor_copy(
            out=v_all[:, t, :, 0:64],
            in_=ps[:].rearrange("p (h d) -> p h d", h=3))

    def proj_chunk(n):
        for m in range(3):
            qk_group(m, n)
        for ii in range(4):
            v_group(ii, n)

    # ---- attention ----
    def KQ(h):
        if h == 0:
            return kA[0:64], qA[0:64]
        if h == 1:
            return kA[64:128], qA[64:128]
        return kB[:], qB[:]

    pts = {}    # (h, q, p) -> pt tile
    pvps = {}   # q -> [per-head psum tile [128, 4, 128] f32 (qc, dims|l)]
    pvst = {}   # (q, h) -> started flag for psum zero-region

    def diag_mask(pt, c0):
        # zero pt[:, c0:c0+128] where key row p > query col offset c
        nc.gpsimd.affine_select(
            out=pt[:, c0:c0 + 128], in_=pt[:, c0:c0 + 128],
            compare_op=GE, fill=0.0,
            base=0, channel_multiplier=-1, pattern=[[1, 128]])

    def qk_unit(h, qq, p):
        """Score tile + exp for unit p of quarter qq, head h."""
        Kh, Qh = KQ(h)
        base = 512 * qq
        nfull = 2 * qq

        def kblk(i):
            return Kh[:, 128 * i:128 * (i + 1)]

        def qrng(a, b):
            return Qh[:, base + a:base + b]

        if p < nfull:
            # full pair: key blocks 2p, 2p+1, all 512 queries
            st = st_tile([128, 1024], f"st{rep}_{h}{qq}{p}")
            if "2" in kdup and h < 2:
                # probe: same matmul on the OPPOSITE row half, spare bank
                Kf, Qf = KQ(1 - h)
                dps = psp.tile([128, 512], F32, name=f"dp{rep}_{h}{qq}{p}",
                               tag="bcp")
                nc.tensor.matmul(dps[:], lhsT=Kf[:, 128 * 2 * p:128 * (2 * p + 1)],
                                 rhs=Qf[:, 512 * qq:512 * qq + 512],
                                 start=True, stop=True, skip_group_check=True)
            for _ in range(2 if "q" in kdup else 1):
                nc.tensor.matmul(st[:, 0:512], lhsT=kblk(2 * p), rhs=qrng(0, 512),
                                 start=True, stop=True, skip_group_check=True)
                nc.tensor.matmul(st[:, 512:1024], lhsT=kblk(2 * p + 1),
                                 rhs=qrng(0, 512), start=True, stop=True,
                                 skip_group_check=True)
            pt = sbp.tile([128, 1024], BF16, name=f"pF{rep}_{h}{qq}{p}",
                          tag=f"pF{h}_{qq % 2}_{p}")
            nc.scalar.activation(out=pt[:], in_=st[:], func=Exp,
                                 scale=float(SCALE))
            if "e" in kdup:
                dmy = sbp.tile([128, 1024], BF16, name=f"dm{rep}_{h}{qq}{p}",
                               tag=f"dm{p % 2}")
                nc.scalar.activation(out=dmy[:], in_=st[:], func=Exp,
                                     scale=float(SCALE))
            pts[(h, qq, p)] = pt
        elif p == nfull:
            # D1: bank A = [A-main 384 | A-diag 128], bank B = [B 384 | C-solo 128]
            iA = 4 * qq
            st = st_tile([128, 1024], f"sd{rep}_{h}{qq}")
            nc.tensor.matmul(st[:, 0:384], lhsT=kblk(iA), rhs=qrng(128, 512),
                             start=True, stop=False)
            nc.tensor.matmul(st[:, 384:512], lhsT=kblk(iA), rhs=qrng(0, 128),
                             start=False, stop=True)
            nc.tensor.matmul(st[:, 512:896], lhsT=kblk(iA + 1), rhs=qrng(128, 512),
                             start=True, stop=False)
            nc.tensor.matmul(st[:, 896:1024], lhsT=kblk(iA + 2), rhs=qrng(256, 384),
                             start=False, stop=True)
            pt = sbp.tile([128, 1024], BF16, name=f"pD1{rep}_{h}{qq}",
                          tag=f"pD1{h}_{qq % 2}")
            nc.scalar.activation(out=pt[:], in_=st[:], func=Exp,
                                 scale=float(SCALE))
            diag_mask(pt, 384)        # A's diagonal (queries 0:128)
            diag_mask(pt, 512)        # B's diagonal (queries 128:256)
            diag_mask(pt, 896)        # C's diagonal (queries 256:384)
            pts[(h, qq, p)] = pt
        else:
            # D2: [C-main 128 | D 128]
            iA = 4 * qq
            st = st_tile([128, 256], f"s2{rep}_{h}{qq}")
            nc.tensor.matmul(st[:, 0:128], lhsT=kblk(iA + 2), rhs=qrng(384, 512),
                             start=True, stop=False)
            nc.tensor.matmul(st[:, 128:256], lhsT=kblk(iA + 3), rhs=qrng(384, 512),
                             start=False, stop=True)
            pt = sbp.tile([128, 256], BF16, name=f"pD2{rep}_{h}{qq}",
                          tag=f"pD2{h}_{qq % 2}")
            nc.scalar.activation(out=pt[:], in_=st[:], func=Exp,
                                 scale=float(SCALE))
            diag_mask(pt, 128)        # D's diagonal (queries 384:512)
            pts[(h, qq, p)] = pt

    def pv_unit(h, pvq, p):
        """Flipped PV: pt chunks are STATIONARY ([128 keys, 128 queries]),
        v_all [128, 65] streams -> psum [128 queries, 4qc, 65] per head.
        Streams 65 cols/block-chunk instead of up-to-512: PV runs at the
        output-element bound (8,840 cycles/head vs 17,408)."""
        pvp = pvps[pvq][h]
        nfull = 2 * pvq
        pt = pts.pop((h, pvq, p))

        def mm(qc, c0, i, stop=False):
            st = not pvst.get((pvq, h), False)
            pvst[(pvq, h)] = True
            nc.tensor.matmul(pvp[:, qc, 0:65], lhsT=pt[:, c0:c0 + 128],
                             rhs=v_all[:, i, h, 0:65], start=st, stop=stop)

        if p < nfull:
            for j in range(2):
                for qc in range(4):
                    mm(qc, 512 * j + 128 * qc, 2 * p + j)
        elif p == nfull:
            iA = 4 * pvq
            for qc in (1, 2, 3):
                mm(qc, 128 * (qc - 1), iA)              # A-main
            mm(0, 384, iA)                              # A-diag
            for qc in (1, 2, 3):
                mm(qc, 512 + 128 * (qc - 1), iA + 1)    # B
            mm(2, 896, iA + 2)                          # C-solo
        else:
            iA = 4 * pvq
            mm(3, 0, iA + 2)                            # C-main
            mm(3, 128, iA + 3, stop=True)               # D

    def normalize(q):
        """l sits on the 65th column per query-partition: reciprocal +
        per-partition scale on DVE, then PE-transpose back to dim-major
        ot01/ot2 for the output projection."""
        pvp = pvps.pop(q)
        otns, rcs = [], []
        for qc in range(4):
            t = 4 * q + qc
            rc = sbp.tile([128, 3], F32, name=f"rc{rep}_{t}", tag=f"rc{qc}")
            otn = sbp.tile([128, 3, 64], BF16, name=f"on{rep}_{t}",
                           tag=f"on{qc}")
            for h in range(3):
                with nc.allow_low_precision(reason="f32 recip"):
                    nc.vector.reciprocal(out=rc[:, h:h + 1],
                                         in_=pvp[h][:, qc, 64:65])
            rcs.append(rc)
            otns.append(otn)
        for qc in range(4):
            for h in range(3):
                nc.vector.tensor_scalar_mul(out=otns[qc][:, h, :],
                                            in0=pvp[h][:, qc, 0:64],
                                            scalar1=rcs[qc][:, h:h + 1])
        for qc in range(4):
            t = 4 * q + qc
            tsl = slice(128 * t, 128 * (t + 1))
            dsts = [ot01[0:64, tsl], ot01[64:128, tsl], ot2[0:64, tsl]]
            for h in range(3):
                tp = psp.tile([64, 128], BF16, name=f"tp{rep}_{t}{h}",
                              tag=f"pv{h}")
                nc.tensor.transpose(tp[:], otns[qc][:, h, :], ident_b[:])
                nc.vector.tensor_copy(out=dsts[h], in_=tp[:])

    def phase3_tile(i):
        # out[128i:128i+128, :] = [ot01; ot2][:, tile i].T @ wf
        # Two sequential passes through the bcp bank (no pv-tag use, so the
        # tiles can interleave INTO the exp-paced streams where PE has slack
        # and DVE is idle).
        csl = slice(128 * i, 128 * (i + 1))
        ob = sbp.tile([128, EMBED_DIM], F16, name=f"ob{rep}_{i}",
                      tag=f"ob{i % 3}")
        for (c0, c1) in ((0, 512), (512, 768)):
            fp = psp.tile([128, c1 - c0], F32, name=f"fp{rep}_{i}_{c0}",
                          tag="bcp")
            nc.tensor.matmul(fp[:], lhsT=ot01[:, csl],
                             rhs=wf01_sb[:, c0:c1], start=True, stop=False)
            nc.tensor.matmul(fp[:], lhsT=ot2[:, csl],
                             rhs=wf2_sb[:, c0:c1], start=False, stop=True)
            nc.vector.tensor_copy(out=ob[:, c0:c1], in_=fp[:])
        dmae[i % 2].dma_start(out=out_d[128 * i:128 * (i + 1), :], in_=ob[:])

    # ---- emission closures ----
    def prologue():
        emit_xt_dma()
        for m in range(3):
            qk_group(m, 0)
        for p in range(2):          # quarter 0 bootstrap: D1, D2
            for h in range(3):
                qk_unit(h, 0, p)
            v_group(2 * p, 0)
            v_group(2 * p + 1, 0)

    def main(next_prologue=None):
        # normalize(q)/phase3(q) are deferred into step q+1, AFTER its
        # projection chunk: the proj matmuls sit ahead of the recip/bcast
        # chain in the in-order PE queue, hiding the chain's DVE latency.
        pending = None
        for q in range(4):
            if q == 3 and next_prologue is not None:
                next_prologue()
            if q < 3:
                for m in range(3):
                    qk_group(m, q + 1)      # feeds THIS stream's QK
            if pending is not None:
                normalize(pending)
            qq = q + 1 if q < 3 else None
            pvps[q] = [psp.tile([128, 4, 128], F32, name=f"pv{rep}_{q}{h}",
                                tag=f"pv{h}") for h in range(3)]
            nqk = 2 * (q + 1) + 2 if qq is not None else 0
            npv = 2 * q + 2
            for p in range(max(nqk, npv)):
                for h in range(3):
                    if p < nqk:
                        qk_unit(h, qq, p)
                    if p < npv:
                        pv_unit(h, q, p)
                if pending is not None and p < 4:
                    # in the q=3 (PV-only) stream the interleaved PV units
                    # absorb the fp->ob-copy ping-pong waits
                    phase3_tile(4 * pending + p)
                if q < 3 and p < 4:
                    # chunk q+1's V rides in-stream: its psum drains via an
                    # idle-DVE copy, and PV(q) only reads older v tiles
                    v_group(p, q + 1)
            pending = q
        normalize(3)
        for i in range(12, 16):
            phase3_tile(i)

    return prologue, main


def _prep_inputs(x, w_qkv, b_qkv, w_final):
    """Build the 8 per-core input maps from the full inputs."""
    import ml_dtypes
    fp8 = ml_dtypes.float8_e4m3
    bf16 = ml_dtypes.bfloat16

    x = np.asarray(x, dtype=np.float32)
    w_qkv = np.asarray(w_qkv, dtype=np.float32)
    b_qkv = np.asarray(b_qkv, dtype=np.float32)
    w_final = np.asarray(w_final, dtype=np.float32)
    E = EMBED_DIM

    in_maps = []
    for c in range(N_CORES):
        b = c // 4
        g = c % 4
        heads = [3 * g, 3 * g + 1, 3 * g + 2]
        hr = [np.arange(64 * h, 64 * h + 64) for h in heads]
        # [q0 q1 | k0 k1 | q2 | k2]
        rows_qk = np.concatenate([hr[0], hr[1], E + hr[0], E + hr[1], hr[2], E + hr[2]])
        rows_v = np.concatenate(hr) + 2 * E
        xT = np.ascontiguousarray(x[b].T).astype(bf16)               # [768, 2048]
        wqk = np.ascontiguousarray(w_qkv[rows_qk].T).astype(bf16)    # [768, 384]
        wv = np.ascontiguousarray(w_qkv[rows_v].T).astype(bf16)      # [768, 192]
        bqk = np.ascontiguousarray(b_qkv[rows_qk][:, None])
        wf = np.ascontiguousarray(w_final[:, np.concatenate(hr)].T).astype(bf16)
        in_maps.append({"xT": xT, "wqk": wqk, "wv": wv, "bqk": bqk, "wf": wf})
    return in_maps


def kernel(x, w_qkv, b_qkv, w_final, _trace=False):
    from concourse.bass_utils import run_bass_kernel_spmd

    if "nc" not in _state:
        _state["nc"] = _build()
    nc = _state["nc"]

    in_maps = _prep_inputs(x, w_qkv, b_qkv, w_final)
    res = run_bass_kernel_spmd(nc, in_maps, list(range(N_CORES)), trace=_trace)
    _state["last_result"] = res

    w_final = np.asarray(w_final, dtype=np.float64)
    b_qkv = np.asarray(b_qkv, dtype=np.float64)
    # v bias folds into a constant row: softmax weights sum to 1
    const_row = w_final @ b_qkv[2 * EMBED_DIM:]

    out = np.empty((B, T, EMBED_DIM), dtype=np.float32)
    for b in range(B):
        acc = np.zeros((T, EMBED_DIM), dtype=np.float64)
        for g in range(4):
            acc += res.results[4 * b + g]["out_p"].astype(np.float64)
        out[b] = (acc + const_row).astype(np.float32)
    return out
